# revision 13
# baseline (speedup 1.0000x reference)
"""Trainium2 Bass kernel for nn_Capsule (dynamic routing), bf16 dataflow.

reference: u = x @ W  (per-sample [512,256]@[256,512]); b=0
           3x { c = softmax_o(b); v[o,f] = sum_i c[o,i] u[i,(o,f)];
                v = squash(v); b[o,i] = sum_f v[o,f] u[i,(o,f)] }
           return v [B, 32, 16]

u is never materialized.  Per core: 16 samples = 4 quads of 4.
All matmuls in bf16 (1 cyc/row, fast LDWEIGHTS); fp32 only in PSUM and
the squash scalar chain.  Host ships x twice (natural + pre-transposed)
in bf16, so no on-device transposes of x are needed.

Per (iter t, quad q), layouts ([partition, free]):
  cT   [i%128, (ic4, s4, o32)]  bf16   (t=0: uniform 1/32)
  y    = cT.T @ x          -> ps_y  [(s,o)128, h256]      (16 MM ap256)
  yT   via 2 PE transposes -> yt_sb [h%128, (hc2, so128)] bf16
  vfT  = W @ yT            -> ps_vf [of%128, (g4, so128)] (8 MM ap128)
  diag: mask-mul (gpsimd) + free-reduce over o' (DVE) -> vr [of%128,(g4,s4)] f32
  mag  = I16.T @ vr^2 (PE) -> [o-sub 8, (g,s)16]; factor = exp(.5 ln m)/(1+m)
         (Ln+Exp share one act-table set -> no table reloads)
  fac128 = E8.T @ factor (PE broadcast over f partitions)
  vsq  = vr * fac128 -> bf16
  VmatT: vsq bcast over o' * mask (gpsimd) -> vp [of%128, (g4,s4,o'32)] bf16
  zT   = WT @ VmatT        -> ps_zt [h%128, (hc2, so128)] (8 MM ap128)
  b    = zt.T @ xT         -> ps_b  [(s,o)128, i512]      (8 MM ap512)
  eb   = exp(b) (Act, bf16); ebT via 4 PE transposes (bf16 PSUM)
  softmax over o in [i, (ic,s,o)] layout -> cT for next iter
Last iter stops after vsq; output transposed on PE and cast bf16->f32
by a gpsimd DMA.
"""

import numpy as np
import ml_dtypes

import concourse.bass as bass
import concourse.tile as tile
from concourse import mybir
from concourse.bass_utils import run_bass_kernel_spmd

F32 = mybir.dt.float32
BF16 = mybir.dt.bfloat16
F8 = mybir.dt.float8e4
ZSC = 16.0            # z prescale before fp8 quant (undone in eb's exp)
AF = mybir.ActivationFunctionType
AX = mybir.AxisListType

B, I, H = 128, 512, 256
O, F = 32, 16
OF = O * F            # 512
NCORES = 8
S = B // NCORES       # 16 samples per core
Q = 4                 # quads per core
QS = 4                # samples per quad
NIT = 3
P = 128

# bf16 constant blob [128, CSTN]
CW = 0                # W  [h%128, (hc2, of512)]
CWT = CW + 2 * OF     # WT [of%128, (m4, h256)]
CID = CWT + 4 * H     # identity [128, 128]
CMD = CID + P         # diag mask [128, (g4, o32)]: md[p,(g,o)] = (o == g*8+p//16)
CI16 = CMD + 4 * O    # [128, 8]: i16[p, j] = (j == p//16)
CE8 = CI16 + 8        # [8, 128]: e8[j, p] = (p//16 == j)
CC0 = CE8 + P         # [128, 32] = 1/32
CIDF = CC0 + O        # 2 bf16 cols whose bytes alias to f32 1.0
CC0E = CIDF + 2       # [128, 16]: c0e[i, 4s+j] = (j==s)/32  (t0 A lhsT)
CSTN = CC0E + 16


def ap(t, dims, off=0):
    """AP over tile/handle `t`: keep partition dim, explicit free dims."""
    a = t if isinstance(t, bass.AP) else t[:]
    return bass.AP(tensor=a.tensor, offset=a.offset + off,
                   ap=[list(a.ap[0])] + [list(d) for d in dims])


def f32view(a, off=0):
    """1x1 f32 alias of an SBUF AP's base (for sync-anchor reads only)."""
    t = a.tensor
    t2 = t if t.dtype == F32 else bass.SBTensorHandle(
        name=t.name, shape=[t.shape[0], t.shape[1] // 2], dtype=F32,
        base_partition=t.base_partition,
        manual_sbuf_range=t.manual_sbuf_range,
        manual_base_name=t.manual_base_name)
    return bass.AP(tensor=t2, offset=off,
                   ap=[[int(t2.shape[1]), 1], [1, 1]])


def dram_ap(handle, dims, off=0):
    """AP over DRAM handle with fully explicit dims (first = partition)."""
    a = handle[:]
    return bass.AP(tensor=a.tensor, offset=a.offset + off,
                   ap=[list(d) for d in dims])


MM_LABELS = []


def build_program(split_waits=True):
    MM_LABELS.clear()
    nc = bass.Bass("TRN2", target_bir_lowering=False)

    x_d = nc.dram_tensor("xb", [Q, P, QS * 4 * H], BF16, kind="ExternalInput")
    xt_d = nc.dram_tensor("xtb", [Q, P, QS * 2 * I], BF16, kind="ExternalInput")
    cst_d = nc.dram_tensor("cst", [P, CSTN], BF16, kind="ExternalInput")
    out_d = nc.dram_tensor("out", [S, OF], F32, kind="ExternalOutput")

    with tile.TileContext(nc) as tc:
        with (
            tc.tile_pool(name="consts", bufs=1) as consts,
            tc.tile_pool(name="xpool", bufs=4) as xpool,
            tc.tile_pool(name="xtpool", bufs=4) as xtpool,
            tc.tile_pool(name="work", bufs=2) as work,
            tc.tile_pool(name="ps", bufs=1, space="PSUM") as ps,
        ):
            cst = consts.tile([P, CSTN], BF16)

            def load_cst(c0, c1):
                nc.sync.dma_start(
                    out=ap(cst, [[1, c1 - c0]], off=c0),
                    in_=dram_ap(cst_d, [[CSTN, P], [1, c1 - c0]], off=c0),
                )
            w_sb = cst[:, CW:CW + 2 * OF]
            wt_sb = cst[:, CWT:CWT + 4 * H]
            id_sb = cst[:, CID:CID + P]
            md_sb = cst[:, CMD:CMD + 4 * O]
            i16_sb = cst[:, CI16:CI16 + 8]
            e8_sb = cst[:8, CE8:CE8 + P]
            c0_sb = cst[:, CC0:CC0 + O]
            c0e_sb = cst[:, CC0E:CC0E + 16]

            # PE sync anchors: PE observes foreign engine clocks via 1x1
            # transposes so walrus can elide per-instruction waits.
            anch = None  # anchors disabled; bank freed for the vo tile
            idf1 = f32view(cst[:], off=CIDF // 2)
            dirty = {}
            acol = [0]
            pending = []

            def mark(key, apv):
                dirty[key] = apv

            def pe_sync(*keys, force=False):
                # Anchors proved to over-serialize (PE waits on the globally
                # newest op of an engine, not the actual dependency); rely on
                # Tile's precise per-instruction deps instead.
                pending.clear()
                if not force:
                    return
                for k in keys:
                    if k not in dirty:
                        continue
                    d = dirty.pop(k)
                    MM_LABELS.append("anch:" + k)
                    a = nc.tensor.transpose(
                        anch[:1, acol[0]:acol[0] + 1], f32view(d),
                        idf1)
                    pending.append(a.ins)
                    acol[0] = (acol[0] + 1) % 16

            def _chain(b):
                for a in pending:
                    bass._add_dep_helper(b.ins, a, sync=False,
                                         reason="pe-anchor order")
                return b

            def T(out, in_, ident, label="", **kw):
                MM_LABELS.append(label or CUR[0] + ":T")
                return _chain(nc.tensor.transpose(out, in_, ident, **kw))

            def MM(out, lhsT, rhs, label="", **kw):
                MM_LABELS.append(label or CUR[0])
                return _chain(nc.tensor.matmul(out, lhsT, rhs, **kw))

            mark("cst", cst)
            CUR = ["init"]

            # ---- input DMAs (x natural + pre-transposed, interleaved so
            # quad q's xT lands before its first b-stage) ----
            x_sb = {}
            xt_sb = {}

            NXW = QS * 4 * H

            def load_x(q, halves=1):
                xs = xpool.tile([P, NXW], BF16, tag="x", name="xq%d" % q)
                hw_ = NXW // halves
                for h in range(halves):
                    nc.sync.dma_start(
                        out=ap(xs, [[1, hw_]], off=h * hw_),
                        in_=dram_ap(x_d, [[NXW, P], [1, hw_]],
                                    off=q * P * NXW + h * hw_),
                    )
                mark("x%d" % q, xs)
                x_sb[q] = xs

            def load_xt(q):
                xt = xtpool.tile([P, NXW], BF16, tag="xt")
                nc.sync.dma_start(
                    out=ap(xt, [[1, NXW]]),
                    in_=dram_ap(xt_d, [[NXW, P], [1, NXW]], off=q * P * NXW),
                )
                mark("xt%d" % q, xt)
                xt_sb[q] = xt

            # JIT order: small consts, x0, W, x1, xt0, x2, xt1, x3, xt2, xt3
            load_cst(CID, CSTN)
            load_x(0)
            load_cst(CW, CID)
            load_x(1)
            load_xt(0)
            load_x(2)
            load_xt(1)
            load_x(3)
            load_xt(2)
            load_xt(3)

            # PE p-state warmup: ~14 dummy matmuls on early consts while
            # the x0 DMA streams.  The PE clock ramps to full speed after
            # ~3us of continuous execution (pstate low/mid otherwise), so
            # burning the DMA-wait keeps the real A-stage at full clock.
            ps_w = ps.tile([P, H], F32, tag="y", name="ps_warm")
            for _wi in range(16):
                MM(ps_w[:, :H], id_sb,
                   ap(cst, [[1, H]], off=CID), label="warm")

            cT = {q: None for q in range(Q)}
            fin = {}

            def mk_chunks(q):
                """Per-quad list of emission chunks; wavefront-interleaved
                across quads so PE always has another quad's work during
                cross-engine chain latencies."""
                st = {}
                chunks = []
                for t in range(NIT):
                    chunks.extend(stage_fns(q, t, st))
                return chunks

            def stage_fns(q, t, st):
                last = (t == NIT - 1)

                def A():
                    CUR[0] = "A.q%d.t%d" % (q, t)
                    pe_sync("cst", "x%d" % q, "dve")
                    ps_y = ps.tile([P, H], F32, tag="y")
                    if t == 0:
                        # uniform c: y0 is o-independent -> [4, 256] rows,
                        # one 16-MM accumulation group via c0e selectors
                        for s in range(QS):
                            for ic in range(4):
                                MM(
                                    ps_y[0:4, :],
                                    ap(c0e_sb, [[1, 4]], off=4 * s),
                                    ap(x_sb[q], [[1, H]],
                                       off=s * 4 * H + ic * H),
                                    start=(s == 0 and ic == 0),
                                    stop=(s == 3 and ic == 3),
                                )
                        ysb = work.tile([4, H], BF16, tag="ysb0", bufs=4)
                        nc.scalar.activation(ysb[:], ps_y[0:4, :], AF.Copy)
                    else:
                        for s in range(QS):
                            for ic in range(4):
                                MM(
                                    ps_y[32 * s:32 * s + 32, :],
                                    ap(cT[q], [[1, O]], off=ic * P + s * O),
                                    ap(x_sb[q], [[1, H]],
                                       off=s * 4 * H + ic * H),
                                    start=(ic == 0),
                                    stop=(ic == 3),
                                    tile_position=(0, 32 * s),
                                )
                        ysb = work.tile([P, H], BF16, tag="ysb", bufs=4)
                        nc.scalar.activation(ysb[:], ps_y[:], AF.Copy)
                    mark("act", ysb)
                    st["ysb"] = ysb

                def Bc():
                    CUR[0] = "Bc.q%d.t%d" % (q, t)
                    pe_sync("act")
                    ps_yt = ps.tile([P, 4 * P], BF16, tag="ytb")
                    if t == 0:
                        for hc in range(2):
                            T(ps_yt[:, hc * 4:(hc + 1) * 4],
                              st["ysb"][:4, hc * P:(hc + 1) * P],
                              id_sb[:4, :4])
                        ytsb = work.tile([P, 8], BF16, tag="ytsb0", bufs=4)
                        nc.scalar.activation(ytsb[:], ps_yt[:, :8], AF.Copy)
                    else:
                        for hc in range(2):
                            T(ps_yt[:, hc * P:(hc + 1) * P],
                              st["ysb"][:, hc * P:(hc + 1) * P], id_sb)
                        ytsb = work.tile([P, 2 * P], BF16, tag="ytsb", bufs=4)
                        nc.scalar.activation(ytsb[:], ps_yt[:, :2 * P],
                                             AF.Copy)
                    mark("act", ytsb)
                    st["ytsb"] = ytsb

                def C():
                    CUR[0] = "C.q%d.t%d" % (q, t)
                    pe_sync("dve")
                    if t == 0:
                        ps_vf = ps.tile([P, 16], F32, tag="vf",
                                        name="ps_vf0")
                        for g in range(4):
                            for hc in range(2):
                                MM(
                                    ps_vf[:, g * 4:(g + 1) * 4],
                                    ap(w_sb, [[1, P]], off=hc * OF + g * P),
                                    ap(st["ytsb"], [[1, 4]], off=hc * 4),
                                    start=(hc == 0),
                                    stop=(hc == 1),
                                )
                        vr = work.tile([P, 16], F32, tag="vr", bufs=4)
                        nc.vector.tensor_copy(vr[:], ps_vf[:])
                    else:
                        ps_vf = ps.tile([P, 4 * P], F32, tag="vf")
                        for g in range(4):
                            for hc in range(2):
                                MM(
                                    ps_vf[:, g * P:(g + 1) * P],
                                    ap(w_sb, [[1, P]], off=hc * OF + g * P),
                                    ap(st["ytsb"], [[1, P]], off=hc * P),
                                    start=(hc == 0),
                                    stop=(hc == 1),
                                )
                        msk = work.tile([P, 4 * P], BF16, tag="msk")
                        nc.vector.tensor_mul(
                            ap(msk, [[P, 4], [O, QS], [1, O]]),
                            ap(ps_vf, [[P, 4], [O, QS], [1, O]]),
                            ap(md_sb, [[O, 4], [0, QS], [1, O]]),
                        )
                        st["msk"] = msk
                        vr = work.tile([P, 16], F32, tag="vr", bufs=4)
                        nc.vector.reduce_sum(
                            out=vr[:],
                            in_=ap(msk, [[O, 16], [1, O]]),
                            axis=AX.X,
                        )
                    vrsq = work.tile([P, 16], BF16, tag="vrsq", bufs=4)
                    nc.vector.tensor_mul(vrsq[:], vr[:], vr[:])
                    mark("dve", vrsq)
                    st["vr"] = vr
                    st["vrsq"] = vrsq

                def D():
                    CUR[0] = "D.q%d.t%d" % (q, t)
                    pe_sync("dve")
                    mf = ps.tile([P, 32], F32, tag="mf")
                    if t > 0:
                        MM(mf[:8, :16], i16_sb,
                           ap(st["msk"], [[1, 16]]), label="ping")
                    MM(mf[:8, :16], i16_sb, st["vrsq"][:])
                    lnm = work.tile([8, 16], F32, tag="lnm", bufs=4)
                    nc.scalar.activation(lnm[:], mf[:8, :16], AF.Ln)
                    s0 = work.tile([8, 16], F32, tag="s0", bufs=4)
                    nc.scalar.activation(s0[:], lnm[:], AF.Exp, scale=0.5)
                    mark("act", s0)
                    onep = work.tile([8, 16], F32, tag="onep", bufs=4)
                    nc.vector.tensor_scalar_add(onep[:], mf[:8, :16], 1.0)
                    rp = work.tile([8, 16], F32, tag="rp", bufs=4)
                    nc.vector.reciprocal(rp[:], onep[:])
                    facb = work.tile([8, 16], BF16, tag="facb", bufs=4)
                    nc.vector.tensor_mul(facb[:], s0[:], rp[:])
                    mark("dve", facb)
                    st["mf"] = mf
                    st["facb"] = facb

                def E():
                    CUR[0] = "E.q%d.t%d" % (q, t)
                    mf = st["mf"]
                    pe_sync("dve", "act")
                    MM(mf[:, 16:32], e8_sb, st["facb"][:8, :])
                    if not last:
                        vsq = work.tile([P, 16], BF16, tag="vsq", bufs=4)
                        nc.vector.tensor_mul(vsq[:], st["vr"][:],
                                             mf[:, 16:32])
                        mark("dve", vsq)
                        vp = work.tile([P, 4 * P], BF16, tag="vp", bufs=4)
                        nc.gpsimd.tensor_mul(
                            ap(vp, [[P, 4], [O, QS], [1, O]]),
                            ap(vsq, [[4, 4], [1, QS], [0, O]]),
                            ap(md_sb, [[O, 4], [0, QS], [1, O]]),
                        )
                        mark("pool", vp)
                        st["vp"] = vp
                    else:
                        # all quads write one [128, (q, s, g)] tile; single
                        # transpose + copy + cast-DMA at the last quad
                        if "vsq_all" not in fin:
                            fin["vsq_all"] = work.tile([P, 64], BF16,
                                                       tag="vsq_all", bufs=1,
                                                       name="vsq_all")
                        nc.vector.tensor_mul(
                            ap(fin["vsq_all"], [[1, 4], [4, 4]], off=16 * q),
                            ap(st["vr"], [[4, 4], [1, 4]]),
                            ap(mf, [[4, 4], [1, 4]], off=16),
                        )
                        fin["done"] = fin.get("done", 0) + 1
                        if fin["done"] == 2:
                            # first output half transposed early, hidden
                            # under the remaining quads' t2 compute
                            fin["ps_vo"] = ps.tile([P, P], BF16, tag="y",
                                                   name="ps_vo")
                            T(fin["ps_vo"][0:32, :P],
                              fin["vsq_all"][:, 0:32], id_sb, label="OUT:T1")
                        if fin["done"] == Q:
                            ps_vo = fin["ps_vo"]
                            T(ps_vo[32:64, :P], fin["vsq_all"][:, 32:64],
                              id_sb, label="OUT:T2")
                            # f32 cast in the DVE copy so the output DMA can
                            # use the sync engine's HW DGE (no gpsimd SW-DGE
                            # cast overhead on the tail)
                            vo = work.tile([64, P], F32, tag="vosb")
                            nc.vector.tensor_copy(vo[:], ps_vo[:64, :P])
                            nc.sync.dma_start(
                                out=dram_ap(out_d, [[P, 64], [1, P]]),
                                in_=vo[:],
                            )

                def Fc():
                    CUR[0] = "Fc.q%d.t%d" % (q, t)
                    pe_sync("dve")
                    ps_zt = ps.tile([P, 2 * P], F32, tag="zt")
                    for hc in range(2):
                        for m in range(4):
                            MM(
                                ps_zt[:, hc * P:(hc + 1) * P],
                                ap(wt_sb, [[1, P]], off=m * H + hc * P),
                                ap(st["vp"], [[1, P]], off=m * P),
                                start=(m == 0),
                                stop=(m == 3),
                            )
                    ztsb = work.tile([P, 2 * P], BF16, tag="ztsb", bufs=4)
                    nc.scalar.activation(ztsb[:], ps_zt[:], AF.Copy)
                    mark("act", ztsb)
                    st["ztsb"] = ztsb

                def G():
                    CUR[0] = "G.q%d.t%d" % (q, t)
                    pe_sync("act", "xt%d" % q)
                    ps_b = ps.tile([P, I], F32, tag="b", bufs=2)
                    for s in range(QS):
                        for hc in range(2):
                            MM(
                                ps_b[32 * s:32 * s + 32, :],
                                ap(st["ztsb"], [[1, O]], off=hc * P + s * O),
                                ap(xt_sb[q], [[1, I]], off=s * 2 * I + hc * I),
                                start=(hc == 0),
                                stop=(hc == 1),
                                tile_position=(0, 32 * s),
                            )
                    eb = work.tile([P, I], BF16, tag="eb", bufs=4)
                    nc.scalar.activation(eb[:], ps_b[:], AF.Exp)
                    mark("act", eb)
                    st["eb"] = eb

                def Hc():
                    CUR[0] = "Hc.q%d.t%d" % (q, t)
                    pe_sync("act")
                    ps_ebt = ps.tile([P, 4 * P], BF16, tag="ebt")
                    for ic in range(4):
                        T(ps_ebt[:, ic * P:(ic + 1) * P],
                          st["eb"][:, ic * P:(ic + 1) * P], id_sb)
                    ssum = work.tile([P, 16], F32, tag="ssum")
                    nc.vector.reduce_sum(
                        out=ssum[:],
                        in_=ap(ps_ebt, [[O, 16], [1, O]]),
                        axis=AX.X,
                    )
                    rs = work.tile([P, 16], F32, tag="rs")
                    nc.vector.reciprocal(rs[:], ssum[:])
                    ct = work.tile([P, 4 * P], BF16, tag="ct%d" % q, bufs=1)
                    nc.vector.tensor_mul(
                        ap(ct, [[O, 16], [1, O]]),
                        ap(ps_ebt, [[O, 16], [1, O]]),
                        ap(rs, [[1, 16], [0, O]]),
                    )
                    mark("dve", ct)
                    cT[q] = ct

                if last:
                    return [A, Bc, C, D, E]
                return [A, Bc, C, D, E, Fc, G, Hc]

            all_chunks = {q: mk_chunks(q) for q in range(Q)}
            L = len(all_chunks[0])
            SKEW = 2
            for k in range(L + SKEW * (Q - 1)):
                for q in reversed(range(Q)):
                    c = k - SKEW * q
                    if 0 <= c < L:
                        all_chunks[q][c]()

    if split_waits:
        _split_fat_waits(nc)
    return nc


def _split_fat_waits(nc, maxw=1):
    """Walrus caps sync waits per instruction; split overflow onto extra
    same-engine Drain instructions inserted just before the offender."""
    nsplit = 0
    for blk in nc.m.functions[0].blocks:
        new_insts = []
        for inst in blk.instructions:
            si = getattr(inst, "sync_info", None)
            w = list(si.on_wait) if si is not None and si.on_wait else []
            if len(w) > maxw:
                for k in range(0, len(w) - maxw, maxw):
                    d = mybir.InstDrain(name="I-waitsplit-%d" % nsplit,
                                        ins=[], outs=[])
                    nsplit += 1
                    d.engine = inst.engine
                    d.sync_info = mybir.SyncInfo(on_wait=w[k:k + maxw],
                                                 on_update=[])
                    new_insts.append(d)
                si.on_wait = w[len(w) - maxw:]
            new_insts.append(inst)
        blk.instructions[:] = new_insts
    return nc


_NC_CACHE = None


def make_cst(Wn):
    """bf16 constant blob [128, CSTN] matching the device-side layout."""
    cst = np.zeros((P, CSTN), np.float32)
    cst[:, CW:CW + 2 * OF] = (
        Wn.reshape(2, P, OF).transpose(1, 0, 2).reshape(P, 2 * OF))
    cst[:, CWT:CWT + 4 * H] = (
        Wn.T.reshape(4, P, H).transpose(1, 0, 2).reshape(P, 4 * H))
    cst[:, CID:CID + P] = np.eye(P, dtype=np.float32)
    for p in range(P):
        for g in range(4):
            cst[p, CMD + g * O + g * 8 + p // 16] = 1.0
    cst[np.arange(P), CI16 + np.arange(P) // 16] = 1.0
    for j in range(8):
        cst[j, CE8 + 16 * j:CE8 + 16 * (j + 1)] = 1.0
    cst[:, CC0:CC0 + O] = 1.0 / O
    for s in range(4):
        cst[:, CC0E + 4 * s + s] = 1.0 / O
    out = cst.astype(ml_dtypes.bfloat16)
    # bf16 pair (0.0, 1.0) little-endian == f32 1.0 when viewed 4-byte
    out[:, CIDF] = ml_dtypes.bfloat16(0.0)
    out[:, CIDF + 1] = ml_dtypes.bfloat16(1.0)
    return out


def make_in_maps(x, W):
    x = np.asarray(x, dtype=np.float32)
    Wn = np.asarray(W, dtype=np.float32).reshape(H, OF)
    cst = make_cst(Wn)
    xq = x.astype(ml_dtypes.bfloat16).reshape(NCORES, Q, QS, 4, P, H)
    # xb[c, q, p, (s, ic, h)] = x[c, 4q+s, 128ic+p, h]
    xb = np.ascontiguousarray(xq.transpose(0, 1, 4, 2, 3, 5)).reshape(
        NCORES, Q, P, QS * 4 * H)
    # xtb[c, q, p, (s, hc, i)] = x[c, 4q+s, i, 128hc+p]
    xth = x.astype(ml_dtypes.bfloat16).reshape(NCORES, Q, QS, I, 2, P)
    xtb = np.ascontiguousarray(xth.transpose(0, 1, 5, 2, 4, 3)).reshape(
        NCORES, Q, P, QS * 2 * I)
    return [
        {"xb": xb[c], "xtb": xtb[c], "cst": cst}
        for c in range(NCORES)
    ]


def kernel(x: np.ndarray, W: np.ndarray) -> np.ndarray:
    global _NC_CACHE
    if _NC_CACHE is None:
        _NC_CACHE = build_program()
    in_maps = make_in_maps(x, W)
    res = run_bass_kernel_spmd(_NC_CACHE, in_maps, core_ids=list(range(NCORES)))
    out = np.stack([res.results[c]["out"] for c in range(NCORES)])
    return out.reshape(B, O, F)



# revision 14
# speedup vs baseline: 1.0117x; 1.0117x over previous
"""Trainium2 Bass kernel for nn_Capsule (dynamic routing), bf16 dataflow.

reference: u = x @ W  (per-sample [512,256]@[256,512]); b=0
           3x { c = softmax_o(b); v[o,f] = sum_i c[o,i] u[i,(o,f)];
                v = squash(v); b[o,i] = sum_f v[o,f] u[i,(o,f)] }
           return v [B, 32, 16]

u is never materialized.  Per core: 16 samples = 4 quads of 4.
All matmuls in bf16 (1 cyc/row, fast LDWEIGHTS); fp32 only in PSUM and
the squash scalar chain.  Host ships x twice (natural + pre-transposed)
in bf16, so no on-device transposes of x are needed.

Per (iter t, quad q), layouts ([partition, free]):
  cT   [i%128, (ic4, s4, o32)]  bf16   (t=0: uniform 1/32)
  y    = cT.T @ x          -> ps_y  [(s,o)128, h256]      (16 MM ap256)
  yT   via 2 PE transposes -> yt_sb [h%128, (hc2, so128)] bf16
  vfT  = W @ yT            -> ps_vf [of%128, (g4, so128)] (8 MM ap128)
  diag: mask-mul (gpsimd) + free-reduce over o' (DVE) -> vr [of%128,(g4,s4)] f32
  mag  = I16.T @ vr^2 (PE) -> [o-sub 8, (g,s)16]; factor = exp(.5 ln m)/(1+m)
         (Ln+Exp share one act-table set -> no table reloads)
  fac128 = E8.T @ factor (PE broadcast over f partitions)
  vsq  = vr * fac128 -> bf16
  VmatT: vsq bcast over o' * mask (gpsimd) -> vp [of%128, (g4,s4,o'32)] bf16
  zT   = WT @ VmatT        -> ps_zt [h%128, (hc2, so128)] (8 MM ap128)
  b    = zt.T @ xT         -> ps_b  [(s,o)128, i512]      (8 MM ap512)
  eb   = exp(b) (Act, bf16); ebT via 4 PE transposes (bf16 PSUM)
  softmax over o in [i, (ic,s,o)] layout -> cT for next iter
Last iter stops after vsq; output transposed on PE and cast bf16->f32
by a gpsimd DMA.
"""

import numpy as np
import ml_dtypes

import concourse.bass as bass
import concourse.tile as tile
from concourse import mybir
from concourse.bass_utils import run_bass_kernel_spmd

F32 = mybir.dt.float32
BF16 = mybir.dt.bfloat16
F8 = mybir.dt.float8e4
ZSC = 16.0            # z prescale before fp8 quant (undone in eb's exp)
AF = mybir.ActivationFunctionType
AX = mybir.AxisListType

B, I, H = 128, 512, 256
O, F = 32, 16
OF = O * F            # 512
NCORES = 8
S = B // NCORES       # 16 samples per core
Q = 4                 # quads per core
QS = 4                # samples per quad
NIT = 3
P = 128

# bf16 constant blob [128, CSTN]
CW = 0                # W  [h%128, (hc2, of512)]
CWT = CW + 2 * OF     # WT [of%128, (m4, h256)]
CID = CWT + 4 * H     # identity [128, 128]
CMD = CID + P         # diag mask [128, (g4, o32)]: md[p,(g,o)] = (o == g*8+p//16)
CI16 = CMD + 4 * O    # [128, 8]: i16[p, j] = (j == p//16)
CE8 = CI16 + 8        # [8, 128]: e8[j, p] = (p//16 == j)
CC0 = CE8 + P         # [128, 32] = 1/32
CIDF = CC0 + O        # 2 bf16 cols whose bytes alias to f32 1.0
CC0E = CIDF + 2       # [128, 16]: c0e[i, 4s+j] = (j==s)/32  (t0 A lhsT)
CSTN = CC0E + 16


def ap(t, dims, off=0):
    """AP over tile/handle `t`: keep partition dim, explicit free dims."""
    a = t if isinstance(t, bass.AP) else t[:]
    return bass.AP(tensor=a.tensor, offset=a.offset + off,
                   ap=[list(a.ap[0])] + [list(d) for d in dims])


def f32view(a, off=0):
    """1x1 f32 alias of an SBUF AP's base (for sync-anchor reads only)."""
    t = a.tensor
    t2 = t if t.dtype == F32 else bass.SBTensorHandle(
        name=t.name, shape=[t.shape[0], t.shape[1] // 2], dtype=F32,
        base_partition=t.base_partition,
        manual_sbuf_range=t.manual_sbuf_range,
        manual_base_name=t.manual_base_name)
    return bass.AP(tensor=t2, offset=off,
                   ap=[[int(t2.shape[1]), 1], [1, 1]])


def dram_ap(handle, dims, off=0):
    """AP over DRAM handle with fully explicit dims (first = partition)."""
    a = handle[:]
    return bass.AP(tensor=a.tensor, offset=a.offset + off,
                   ap=[list(d) for d in dims])


MM_LABELS = []


def build_program(split_waits=True):
    MM_LABELS.clear()
    nc = bass.Bass("TRN2", target_bir_lowering=False)

    x_d = nc.dram_tensor("xb", [Q, P, QS * 4 * H], BF16, kind="ExternalInput")
    xt_d = nc.dram_tensor("xtb", [Q, P, QS * 2 * I], BF16, kind="ExternalInput")
    cst_d = nc.dram_tensor("cst", [P, CSTN], BF16, kind="ExternalInput")
    out_d = nc.dram_tensor("out", [S, OF], F32, kind="ExternalOutput")

    with tile.TileContext(nc) as tc:
        with (
            tc.tile_pool(name="consts", bufs=1) as consts,
            tc.tile_pool(name="xpool", bufs=4) as xpool,
            tc.tile_pool(name="xtpool", bufs=4) as xtpool,
            tc.tile_pool(name="work", bufs=2) as work,
            tc.tile_pool(name="ps", bufs=1, space="PSUM") as ps,
        ):
            cst = consts.tile([P, CSTN], BF16)

            def load_cst(c0, c1):
                nc.sync.dma_start(
                    out=ap(cst, [[1, c1 - c0]], off=c0),
                    in_=dram_ap(cst_d, [[CSTN, P], [1, c1 - c0]], off=c0),
                )
            w_sb = cst[:, CW:CW + 2 * OF]
            wt_sb = cst[:, CWT:CWT + 4 * H]
            id_sb = cst[:, CID:CID + P]
            md_sb = cst[:, CMD:CMD + 4 * O]
            i16_sb = cst[:, CI16:CI16 + 8]
            e8_sb = cst[:8, CE8:CE8 + P]
            c0_sb = cst[:, CC0:CC0 + O]
            c0e_sb = cst[:, CC0E:CC0E + 16]

            # PE sync anchors: PE observes foreign engine clocks via 1x1
            # transposes so walrus can elide per-instruction waits.
            anch = None  # anchors disabled; bank freed for the vo tile
            idf1 = f32view(cst[:], off=CIDF // 2)
            dirty = {}
            acol = [0]
            pending = []

            def mark(key, apv):
                dirty[key] = apv

            def pe_sync(*keys, force=False):
                # Anchors proved to over-serialize (PE waits on the globally
                # newest op of an engine, not the actual dependency); rely on
                # Tile's precise per-instruction deps instead.
                pending.clear()
                if not force:
                    return
                for k in keys:
                    if k not in dirty:
                        continue
                    d = dirty.pop(k)
                    MM_LABELS.append("anch:" + k)
                    a = nc.tensor.transpose(
                        anch[:1, acol[0]:acol[0] + 1], f32view(d),
                        idf1)
                    pending.append(a.ins)
                    acol[0] = (acol[0] + 1) % 16

            def _chain(b):
                for a in pending:
                    bass._add_dep_helper(b.ins, a, sync=False,
                                         reason="pe-anchor order")
                return b

            def T(out, in_, ident, label="", **kw):
                MM_LABELS.append(label or CUR[0] + ":T")
                return _chain(nc.tensor.transpose(out, in_, ident, **kw))

            def MM(out, lhsT, rhs, label="", **kw):
                MM_LABELS.append(label or CUR[0])
                return _chain(nc.tensor.matmul(out, lhsT, rhs, **kw))

            mark("cst", cst)
            CUR = ["init"]

            # ---- input DMAs (x natural + pre-transposed, interleaved so
            # quad q's xT lands before its first b-stage) ----
            x_sb = {}
            xt_sb = {}

            NXW = QS * 4 * H

            def load_x(q, halves=1):
                xs = xpool.tile([P, NXW], BF16, tag="x", name="xq%d" % q)
                hw_ = NXW // halves
                for h in range(halves):
                    nc.sync.dma_start(
                        out=ap(xs, [[1, hw_]], off=h * hw_),
                        in_=dram_ap(x_d, [[NXW, P], [1, hw_]],
                                    off=q * P * NXW + h * hw_),
                    )
                mark("x%d" % q, xs)
                x_sb[q] = xs

            def load_xt(q):
                xt = xtpool.tile([P, NXW], BF16, tag="xt")
                nc.sync.dma_start(
                    out=ap(xt, [[1, NXW]]),
                    in_=dram_ap(xt_d, [[NXW, P], [1, NXW]], off=q * P * NXW),
                )
                mark("xt%d" % q, xt)
                xt_sb[q] = xt

            # JIT order: small consts, x0, W, x1, xt0, x2, xt1, x3, xt2, xt3
            load_cst(CID, CSTN)
            load_x(0)
            load_cst(CW, CID)
            load_x(1)
            load_xt(0)
            load_x(2)
            load_xt(1)
            load_x(3)
            load_xt(2)
            load_xt(3)

            # PE p-state warmup: ~14 dummy matmuls on early consts while
            # the x0 DMA streams.  The PE clock ramps to full speed after
            # ~3us of continuous execution (pstate low/mid otherwise), so
            # burning the DMA-wait keeps the real A-stage at full clock.
            ps_w = ps.tile([P, H], F32, tag="y", name="ps_warm")
            for _wi in range(16):
                MM(ps_w[:, :H], id_sb,
                   ap(cst, [[1, H]], off=CID), label="warm")

            cT = {q: None for q in range(Q)}
            fin = {}

            def mk_chunks(q):
                """Per-quad list of emission chunks; wavefront-interleaved
                across quads so PE always has another quad's work during
                cross-engine chain latencies."""
                st = {}
                chunks = []
                for t in range(NIT):
                    chunks.extend(stage_fns(q, t, st))
                return chunks

            def stage_fns(q, t, st):
                last = (t == NIT - 1)

                def A():
                    CUR[0] = "A.q%d.t%d" % (q, t)
                    pe_sync("cst", "x%d" % q, "dve")
                    ps_y = ps.tile([P, H], F32, tag="y")
                    if t == 0:
                        # uniform c: y0 is o-independent -> [4, 256] rows,
                        # one 16-MM accumulation group via c0e selectors
                        for s in range(QS):
                            for ic in range(4):
                                MM(
                                    ps_y[0:4, :],
                                    ap(c0e_sb, [[1, 4]], off=4 * s),
                                    ap(x_sb[q], [[1, H]],
                                       off=s * 4 * H + ic * H),
                                    start=(s == 0 and ic == 0),
                                    stop=(s == 3 and ic == 3),
                                )
                        ysb = work.tile([4, H], BF16, tag="ysb0", bufs=4)
                        nc.scalar.activation(ysb[:], ps_y[0:4, :], AF.Copy)
                    else:
                        for s in range(QS):
                            for ic in range(4):
                                MM(
                                    ps_y[32 * s:32 * s + 32, :],
                                    ap(cT[q], [[1, O]], off=ic * P + s * O),
                                    ap(x_sb[q], [[1, H]],
                                       off=s * 4 * H + ic * H),
                                    start=(ic == 0),
                                    stop=(ic == 3),
                                    tile_position=(0, 32 * s),
                                )
                        ysb = work.tile([P, H], BF16, tag="ysb", bufs=4)
                        for hh in range(2):
                            nc.scalar.activation(
                                ysb[:, hh * P:(hh + 1) * P],
                                ps_y[:, hh * P:(hh + 1) * P], AF.Copy)
                    mark("act", ysb)
                    st["ysb"] = ysb

                def Bc():
                    CUR[0] = "Bc.q%d.t%d" % (q, t)
                    pe_sync("act")
                    ps_yt = ps.tile([P, 4 * P], BF16, tag="ytb")
                    if t == 0:
                        for hc in range(2):
                            T(ps_yt[:, hc * 4:(hc + 1) * 4],
                              st["ysb"][:4, hc * P:(hc + 1) * P],
                              id_sb[:4, :4])
                        ytsb = work.tile([P, 8], BF16, tag="ytsb0", bufs=4)
                        nc.scalar.activation(ytsb[:], ps_yt[:, :8], AF.Copy)
                    else:
                        for hc in range(2):
                            T(ps_yt[:, hc * P:(hc + 1) * P],
                              st["ysb"][:, hc * P:(hc + 1) * P], id_sb)
                        ytsb = work.tile([P, 2 * P], BF16, tag="ytsb", bufs=4)
                        nc.scalar.activation(ytsb[:], ps_yt[:, :2 * P],
                                             AF.Copy)
                    mark("act", ytsb)
                    st["ytsb"] = ytsb

                def C():
                    CUR[0] = "C.q%d.t%d" % (q, t)
                    pe_sync("dve")
                    if t == 0:
                        ps_vf = ps.tile([P, 16], F32, tag="vf",
                                        name="ps_vf0")
                        for g in range(4):
                            for hc in range(2):
                                MM(
                                    ps_vf[:, g * 4:(g + 1) * 4],
                                    ap(w_sb, [[1, P]], off=hc * OF + g * P),
                                    ap(st["ytsb"], [[1, 4]], off=hc * 4),
                                    start=(hc == 0),
                                    stop=(hc == 1),
                                )
                        vr = work.tile([P, 16], F32, tag="vr", bufs=4)
                        nc.vector.tensor_copy(vr[:], ps_vf[:])
                    else:
                        ps_vf = ps.tile([P, 4 * P], F32, tag="vf")
                        for g in range(4):
                            for hc in range(2):
                                MM(
                                    ps_vf[:, g * P:(g + 1) * P],
                                    ap(w_sb, [[1, P]], off=hc * OF + g * P),
                                    ap(st["ytsb"], [[1, P]], off=hc * P),
                                    start=(hc == 0),
                                    stop=(hc == 1),
                                )
                        msk = work.tile([P, 4 * P], BF16, tag="msk")
                        nc.vector.tensor_mul(
                            ap(msk, [[P, 4], [O, QS], [1, O]]),
                            ap(ps_vf, [[P, 4], [O, QS], [1, O]]),
                            ap(md_sb, [[O, 4], [0, QS], [1, O]]),
                        )
                        st["msk"] = msk
                        vr = work.tile([P, 16], F32, tag="vr", bufs=4)
                        nc.vector.reduce_sum(
                            out=vr[:],
                            in_=ap(msk, [[O, 16], [1, O]]),
                            axis=AX.X,
                        )
                    vrsq = work.tile([P, 16], BF16, tag="vrsq", bufs=4)
                    nc.vector.tensor_mul(vrsq[:], vr[:], vr[:])
                    mark("dve", vrsq)
                    st["vr"] = vr
                    st["vrsq"] = vrsq

                def D():
                    CUR[0] = "D.q%d.t%d" % (q, t)
                    pe_sync("dve")
                    mf = ps.tile([P, 32], F32, tag="mf")
                    if t > 0:
                        MM(mf[:8, :16], i16_sb,
                           ap(st["msk"], [[1, 16]]), label="ping")
                    MM(mf[:8, :16], i16_sb, st["vrsq"][:])
                    lnm = work.tile([8, 16], F32, tag="lnm", bufs=4)
                    nc.scalar.activation(lnm[:], mf[:8, :16], AF.Ln)
                    s0 = work.tile([8, 16], F32, tag="s0", bufs=4)
                    nc.scalar.activation(s0[:], lnm[:], AF.Exp, scale=0.5)
                    mark("act", s0)
                    onep = work.tile([8, 16], F32, tag="onep", bufs=4)
                    nc.vector.tensor_scalar_add(onep[:], mf[:8, :16], 1.0)
                    rp = work.tile([8, 16], F32, tag="rp", bufs=4)
                    nc.vector.reciprocal(rp[:], onep[:])
                    facb = work.tile([8, 16], BF16, tag="facb", bufs=4)
                    nc.vector.tensor_mul(facb[:], s0[:], rp[:])
                    mark("dve", facb)
                    st["mf"] = mf
                    st["facb"] = facb

                def E():
                    CUR[0] = "E.q%d.t%d" % (q, t)
                    mf = st["mf"]
                    pe_sync("dve", "act")
                    MM(mf[:, 16:32], e8_sb, st["facb"][:8, :])
                    if not last:
                        vsq = work.tile([P, 16], BF16, tag="vsq", bufs=4)
                        nc.vector.tensor_mul(vsq[:], st["vr"][:],
                                             mf[:, 16:32])
                        mark("dve", vsq)
                        vp = work.tile([P, 4 * P], BF16, tag="vp", bufs=4)
                        nc.gpsimd.tensor_mul(
                            ap(vp, [[P, 4], [O, QS], [1, O]]),
                            ap(vsq, [[4, 4], [1, QS], [0, O]]),
                            ap(md_sb, [[O, 4], [0, QS], [1, O]]),
                        )
                        mark("pool", vp)
                        st["vp"] = vp
                    else:
                        # all quads write one [128, (q, s, g)] tile; single
                        # transpose + copy + cast-DMA at the last quad
                        if "vsq_all" not in fin:
                            fin["vsq_all"] = work.tile([P, 64], BF16,
                                                       tag="vsq_all", bufs=1,
                                                       name="vsq_all")
                        nc.vector.tensor_mul(
                            ap(fin["vsq_all"], [[1, 4], [4, 4]], off=16 * q),
                            ap(st["vr"], [[4, 4], [1, 4]]),
                            ap(mf, [[4, 4], [1, 4]], off=16),
                        )
                        fin["done"] = fin.get("done", 0) + 1
                        if fin["done"] == 2:
                            # first output half transposed early, hidden
                            # under the remaining quads' t2 compute
                            fin["ps_vo"] = ps.tile([P, P], BF16, tag="y",
                                                   name="ps_vo")
                            T(fin["ps_vo"][0:32, :P],
                              fin["vsq_all"][:, 0:32], id_sb, label="OUT:T1")
                        if fin["done"] == Q:
                            ps_vo = fin["ps_vo"]
                            T(ps_vo[32:64, :P], fin["vsq_all"][:, 32:64],
                              id_sb, label="OUT:T2")
                            # f32 cast in the DVE copy so the output DMA can
                            # use the sync engine's HW DGE (no gpsimd SW-DGE
                            # cast overhead on the tail)
                            vo = work.tile([64, P], F32, tag="vosb")
                            nc.vector.tensor_copy(vo[:], ps_vo[:64, :P])
                            nc.sync.dma_start(
                                out=dram_ap(out_d, [[P, 64], [1, P]]),
                                in_=vo[:],
                            )

                def Fc():
                    CUR[0] = "Fc.q%d.t%d" % (q, t)
                    pe_sync("dve")
                    ps_zt = ps.tile([P, 2 * P], F32, tag="zt")
                    for hc in range(2):
                        for m in range(4):
                            MM(
                                ps_zt[:, hc * P:(hc + 1) * P],
                                ap(wt_sb, [[1, P]], off=m * H + hc * P),
                                ap(st["vp"], [[1, P]], off=m * P),
                                start=(m == 0),
                                stop=(m == 3),
                            )
                    ztsb = work.tile([P, 2 * P], BF16, tag="ztsb", bufs=4)
                    for hh in range(2):
                        nc.scalar.activation(
                            ztsb[:, hh * P:(hh + 1) * P],
                            ps_zt[:, hh * P:(hh + 1) * P], AF.Copy)
                    mark("act", ztsb)
                    st["ztsb"] = ztsb

                def G():
                    CUR[0] = "G.q%d.t%d" % (q, t)
                    pe_sync("act", "xt%d" % q)
                    ps_b = ps.tile([P, I], F32, tag="b", bufs=2)
                    for s in range(QS):
                        for hc in range(2):
                            MM(
                                ps_b[32 * s:32 * s + 32, :],
                                ap(st["ztsb"], [[1, O]], off=hc * P + s * O),
                                ap(xt_sb[q], [[1, I]], off=s * 2 * I + hc * I),
                                start=(hc == 0),
                                stop=(hc == 1),
                                tile_position=(0, 32 * s),
                            )
                    eb = work.tile([P, I], BF16, tag="eb", bufs=4)
                    for hh in range(2):
                        nc.scalar.activation(
                            eb[:, hh * 2 * P:(hh + 1) * 2 * P],
                            ps_b[:, hh * 2 * P:(hh + 1) * 2 * P], AF.Exp)
                    mark("act", eb)
                    st["eb"] = eb

                def Hc():
                    CUR[0] = "Hc.q%d.t%d" % (q, t)
                    pe_sync("act")
                    ps_ebt = ps.tile([P, 4 * P], BF16, tag="ebt")
                    for ic in range(4):
                        T(ps_ebt[:, ic * P:(ic + 1) * P],
                          st["eb"][:, ic * P:(ic + 1) * P], id_sb)
                    ssum = work.tile([P, 16], F32, tag="ssum")
                    nc.vector.reduce_sum(
                        out=ssum[:],
                        in_=ap(ps_ebt, [[O, 16], [1, O]]),
                        axis=AX.X,
                    )
                    rs = work.tile([P, 16], F32, tag="rs")
                    nc.vector.reciprocal(rs[:], ssum[:])
                    ct = work.tile([P, 4 * P], BF16, tag="ct%d" % q, bufs=1)
                    nc.vector.tensor_mul(
                        ap(ct, [[O, 16], [1, O]]),
                        ap(ps_ebt, [[O, 16], [1, O]]),
                        ap(rs, [[1, 16], [0, O]]),
                    )
                    mark("dve", ct)
                    cT[q] = ct

                if last:
                    return [A, Bc, C, D, E]
                return [A, Bc, C, D, E, Fc, G, Hc]

            all_chunks = {q: mk_chunks(q) for q in range(Q)}
            L = len(all_chunks[0])
            SKEW = 2
            for k in range(L + SKEW * (Q - 1)):
                for q in reversed(range(Q)):
                    c = k - SKEW * q
                    if 0 <= c < L:
                        all_chunks[q][c]()

    if split_waits:
        _split_fat_waits(nc)
    return nc


def _split_fat_waits(nc, maxw=1):
    """Walrus caps sync waits per instruction; split overflow onto extra
    same-engine Drain instructions inserted just before the offender."""
    nsplit = 0
    for blk in nc.m.functions[0].blocks:
        new_insts = []
        for inst in blk.instructions:
            si = getattr(inst, "sync_info", None)
            w = list(si.on_wait) if si is not None and si.on_wait else []
            if len(w) > maxw:
                for k in range(0, len(w) - maxw, maxw):
                    d = mybir.InstDrain(name="I-waitsplit-%d" % nsplit,
                                        ins=[], outs=[])
                    nsplit += 1
                    d.engine = inst.engine
                    d.sync_info = mybir.SyncInfo(on_wait=w[k:k + maxw],
                                                 on_update=[])
                    new_insts.append(d)
                si.on_wait = w[len(w) - maxw:]
            new_insts.append(inst)
        blk.instructions[:] = new_insts
    return nc


_NC_CACHE = None


def make_cst(Wn):
    """bf16 constant blob [128, CSTN] matching the device-side layout."""
    cst = np.zeros((P, CSTN), np.float32)
    cst[:, CW:CW + 2 * OF] = (
        Wn.reshape(2, P, OF).transpose(1, 0, 2).reshape(P, 2 * OF))
    cst[:, CWT:CWT + 4 * H] = (
        Wn.T.reshape(4, P, H).transpose(1, 0, 2).reshape(P, 4 * H))
    cst[:, CID:CID + P] = np.eye(P, dtype=np.float32)
    for p in range(P):
        for g in range(4):
            cst[p, CMD + g * O + g * 8 + p // 16] = 1.0
    cst[np.arange(P), CI16 + np.arange(P) // 16] = 1.0
    for j in range(8):
        cst[j, CE8 + 16 * j:CE8 + 16 * (j + 1)] = 1.0
    cst[:, CC0:CC0 + O] = 1.0 / O
    for s in range(4):
        cst[:, CC0E + 4 * s + s] = 1.0 / O
    out = cst.astype(ml_dtypes.bfloat16)
    # bf16 pair (0.0, 1.0) little-endian == f32 1.0 when viewed 4-byte
    out[:, CIDF] = ml_dtypes.bfloat16(0.0)
    out[:, CIDF + 1] = ml_dtypes.bfloat16(1.0)
    return out


def make_in_maps(x, W):
    x = np.asarray(x, dtype=np.float32)
    Wn = np.asarray(W, dtype=np.float32).reshape(H, OF)
    cst = make_cst(Wn)
    xq = x.astype(ml_dtypes.bfloat16).reshape(NCORES, Q, QS, 4, P, H)
    # xb[c, q, p, (s, ic, h)] = x[c, 4q+s, 128ic+p, h]
    xb = np.ascontiguousarray(xq.transpose(0, 1, 4, 2, 3, 5)).reshape(
        NCORES, Q, P, QS * 4 * H)
    # xtb[c, q, p, (s, hc, i)] = x[c, 4q+s, i, 128hc+p]
    xth = x.astype(ml_dtypes.bfloat16).reshape(NCORES, Q, QS, I, 2, P)
    xtb = np.ascontiguousarray(xth.transpose(0, 1, 5, 2, 4, 3)).reshape(
        NCORES, Q, P, QS * 2 * I)
    return [
        {"xb": xb[c], "xtb": xtb[c], "cst": cst}
        for c in range(NCORES)
    ]


def kernel(x: np.ndarray, W: np.ndarray) -> np.ndarray:
    global _NC_CACHE
    if _NC_CACHE is None:
        _NC_CACHE = build_program()
    in_maps = make_in_maps(x, W)
    res = run_bass_kernel_spmd(_NC_CACHE, in_maps, core_ids=list(range(NCORES)))
    out = np.stack([res.results[c]["out"] for c in range(NCORES)])
    return out.reshape(B, O, F)



# revision 15
# speedup vs baseline: 1.0786x; 1.0662x over previous
"""Trainium2 Bass kernel for nn_Capsule (dynamic routing), bf16 dataflow.

reference: u = x @ W  (per-sample [512,256]@[256,512]); b=0
           3x { c = softmax_o(b); v[o,f] = sum_i c[o,i] u[i,(o,f)];
                v = squash(v); b[o,i] = sum_f v[o,f] u[i,(o,f)] }
           return v [B, 32, 16]

u is never materialized.  Per core: 16 samples = 4 quads of 4.
All matmuls in bf16 (1 cyc/row, fast LDWEIGHTS); fp32 only in PSUM and
the squash scalar chain.  Host ships x twice (natural + pre-transposed)
in bf16, so no on-device transposes of x are needed.

Per (iter t, quad q), layouts ([partition, free]):
  cT   [i%128, (ic4, s4, o32)]  bf16   (t=0: uniform 1/32)
  y    = cT.T @ x          -> ps_y  [(s,o)128, h256]      (16 MM ap256)
  yT   via 2 PE transposes -> yt_sb [h%128, (hc2, so128)] bf16
  vfT  = W @ yT            -> ps_vf [of%128, (g4, so128)] (8 MM ap128)
  diag: mask-mul (gpsimd) + free-reduce over o' (DVE) -> vr [of%128,(g4,s4)] f32
  mag  = I16.T @ vr^2 (PE) -> [o-sub 8, (g,s)16]; factor = exp(.5 ln m)/(1+m)
         (Ln+Exp share one act-table set -> no table reloads)
  fac128 = E8.T @ factor (PE broadcast over f partitions)
  vsq  = vr * fac128 -> bf16
  VmatT: vsq bcast over o' * mask (gpsimd) -> vp [of%128, (g4,s4,o'32)] bf16
  zT   = WT @ VmatT        -> ps_zt [h%128, (hc2, so128)] (8 MM ap128)
  b    = zt.T @ xT         -> ps_b  [(s,o)128, i512]      (8 MM ap512)
  eb   = exp(b) (Act, bf16); ebT via 4 PE transposes (bf16 PSUM)
  softmax over o in [i, (ic,s,o)] layout -> cT for next iter
Last iter stops after vsq; output transposed on PE and cast bf16->f32
by a gpsimd DMA.
"""

import numpy as np
import ml_dtypes

import concourse.bass as bass
import concourse.tile as tile
from concourse import mybir
from concourse.bass_utils import run_bass_kernel_spmd

F32 = mybir.dt.float32
BF16 = mybir.dt.bfloat16
F8 = mybir.dt.float8e4
ZSC = 16.0            # z prescale before fp8 quant (undone in eb's exp)
AF = mybir.ActivationFunctionType
AX = mybir.AxisListType

B, I, H = 128, 512, 256
O, F = 32, 16
OF = O * F            # 512
NCORES = 8
S = B // NCORES       # 16 samples per core
Q = 4                 # quads per core
QS = 4                # samples per quad
NIT = 3
P = 128

# bf16 constant blob [128, CSTN]
CW = 0                # W  [h%128, (hc2, of512)]
CWT = CW + 2 * OF     # WT [of%128, (m4, h256)]
CID = CWT + 4 * H     # identity [128, 128]
CMD = CID + P         # diag mask [128, (g4, o32)]: md[p,(g,o)] = (o == g*8+p//16)
CI16 = CMD + 4 * O    # [128, 8]: i16[p, j] = (j == p//16)
CE8 = CI16 + 8        # [8, 128]: e8[j, p] = (p//16 == j)
CC0 = CE8 + P         # [128, 32] = 1/32
CIDF = CC0 + O        # 2 bf16 cols whose bytes alias to f32 1.0
CC0E = CIDF + 2       # [128, 16]: c0e[i, 4s+j] = (j==s)/32  (t0 A lhsT)
CSTN = CC0E + 16


def ap(t, dims, off=0):
    """AP over tile/handle `t`: keep partition dim, explicit free dims."""
    a = t if isinstance(t, bass.AP) else t[:]
    return bass.AP(tensor=a.tensor, offset=a.offset + off,
                   ap=[list(a.ap[0])] + [list(d) for d in dims])


def f32view(a, off=0):
    """1x1 f32 alias of an SBUF AP's base (for sync-anchor reads only)."""
    t = a.tensor
    t2 = t if t.dtype == F32 else bass.SBTensorHandle(
        name=t.name, shape=[t.shape[0], t.shape[1] // 2], dtype=F32,
        base_partition=t.base_partition,
        manual_sbuf_range=t.manual_sbuf_range,
        manual_base_name=t.manual_base_name)
    return bass.AP(tensor=t2, offset=off,
                   ap=[[int(t2.shape[1]), 1], [1, 1]])


def dram_ap(handle, dims, off=0):
    """AP over DRAM handle with fully explicit dims (first = partition)."""
    a = handle[:]
    return bass.AP(tensor=a.tensor, offset=a.offset + off,
                   ap=[list(d) for d in dims])


MM_LABELS = []


def build_program(split_waits=True):
    MM_LABELS.clear()
    nc = bass.Bass("TRN2", target_bir_lowering=False)

    x_d = nc.dram_tensor("xb", [Q, P, QS * 4 * H], BF16, kind="ExternalInput")
    xt_d = nc.dram_tensor("xtb", [Q, P, QS * 2 * I], BF16, kind="ExternalInput")
    cst_d = nc.dram_tensor("cst", [P, CSTN], BF16, kind="ExternalInput")
    out_d = nc.dram_tensor("out", [S, OF], F32, kind="ExternalOutput")

    with tile.TileContext(nc) as tc:
        with (
            tc.tile_pool(name="consts", bufs=1) as consts,
            tc.tile_pool(name="xpool", bufs=4) as xpool,
            tc.tile_pool(name="xtpool", bufs=4) as xtpool,
            tc.tile_pool(name="work", bufs=2) as work,
            tc.tile_pool(name="ps", bufs=1, space="PSUM") as ps,
        ):
            cst = consts.tile([P, CSTN], BF16)

            def load_cst(c0, c1):
                nc.sync.dma_start(
                    out=ap(cst, [[1, c1 - c0]], off=c0),
                    in_=dram_ap(cst_d, [[CSTN, P], [1, c1 - c0]], off=c0),
                )
            w_sb = cst[:, CW:CW + 2 * OF]
            wt_sb = cst[:, CWT:CWT + 4 * H]
            id_sb = cst[:, CID:CID + P]
            md_sb = cst[:, CMD:CMD + 4 * O]
            i16_sb = cst[:, CI16:CI16 + 8]
            e8_sb = cst[:8, CE8:CE8 + P]
            c0_sb = cst[:, CC0:CC0 + O]
            c0e_sb = cst[:, CC0E:CC0E + 16]

            # PE sync anchors: PE observes foreign engine clocks via 1x1
            # transposes so walrus can elide per-instruction waits.
            anch = None  # anchors disabled; bank freed for the vo tile
            idf1 = f32view(cst[:], off=CIDF // 2)
            dirty = {}
            acol = [0]
            pending = []

            def mark(key, apv):
                dirty[key] = apv

            def pe_sync(*keys, force=False):
                # Anchors proved to over-serialize (PE waits on the globally
                # newest op of an engine, not the actual dependency); rely on
                # Tile's precise per-instruction deps instead.
                pending.clear()
                if not force:
                    return
                for k in keys:
                    if k not in dirty:
                        continue
                    d = dirty.pop(k)
                    MM_LABELS.append("anch:" + k)
                    a = nc.tensor.transpose(
                        anch[:1, acol[0]:acol[0] + 1], f32view(d),
                        idf1)
                    pending.append(a.ins)
                    acol[0] = (acol[0] + 1) % 16

            def _chain(b):
                for a in pending:
                    bass._add_dep_helper(b.ins, a, sync=False,
                                         reason="pe-anchor order")
                return b

            def T(out, in_, ident, label="", **kw):
                MM_LABELS.append(label or CUR[0] + ":T")
                return _chain(nc.tensor.transpose(out, in_, ident, **kw))

            def MM(out, lhsT, rhs, label="", **kw):
                MM_LABELS.append(label or CUR[0])
                return _chain(nc.tensor.matmul(out, lhsT, rhs, **kw))

            mark("cst", cst)
            CUR = ["init"]

            # ---- input DMAs (x natural + pre-transposed, interleaved so
            # quad q's xT lands before its first b-stage) ----
            x_sb = {}
            xt_sb = {}

            NXW = QS * 4 * H

            def load_x(q, halves=1):
                xs = xpool.tile([P, NXW], BF16, tag="x", name="xq%d" % q)
                hw_ = NXW // halves
                for h in range(halves):
                    nc.sync.dma_start(
                        out=ap(xs, [[1, hw_]], off=h * hw_),
                        in_=dram_ap(x_d, [[NXW, P], [1, hw_]],
                                    off=q * P * NXW + h * hw_),
                    )
                mark("x%d" % q, xs)
                x_sb[q] = xs

            def load_xt(q):
                xt = xtpool.tile([P, NXW], BF16, tag="xt")
                nc.sync.dma_start(
                    out=ap(xt, [[1, NXW]]),
                    in_=dram_ap(xt_d, [[NXW, P], [1, NXW]], off=q * P * NXW),
                )
                mark("xt%d" % q, xt)
                xt_sb[q] = xt

            # JIT order: small consts, x0, W, x1, xt0, x2, xt1, x3, xt2, xt3
            load_cst(CID, CSTN)
            load_x(0)
            load_cst(CW, CID)
            load_x(1)
            load_xt(0)
            load_x(2)
            load_xt(1)
            load_x(3)
            load_xt(2)
            load_xt(3)

            # PE p-state warmup: ~14 dummy matmuls on early consts while
            # the x0 DMA streams.  The PE clock ramps to full speed after
            # ~3us of continuous execution (pstate low/mid otherwise), so
            # burning the DMA-wait keeps the real A-stage at full clock.
            ps_w = ps.tile([P, H], F32, tag="y", name="ps_warm")
            for _wi in range(16):
                MM(ps_w[:, :H], id_sb,
                   ap(cst, [[1, H]], off=CID), label="warm")

            cT = {q: None for q in range(Q)}
            fin = {}

            def mk_chunks(q):
                """Per-quad list of emission chunks; wavefront-interleaved
                across quads so PE always has another quad's work during
                cross-engine chain latencies."""
                st = {}
                chunks = []
                for t in range(NIT):
                    chunks.extend(stage_fns(q, t, st))
                return chunks

            def stage_fns(q, t, st):
                last = (t == NIT - 1)

                def A():
                    CUR[0] = "A.q%d.t%d" % (q, t)
                    pe_sync("cst", "x%d" % q, "dve")
                    ps_y = ps.tile([P, H], F32, tag="y")
                    if t == 0:
                        # uniform c: y0 is o-independent -> [4, 256] rows,
                        # one 16-MM accumulation group via c0e selectors
                        for s in range(QS):
                            for ic in range(4):
                                MM(
                                    ps_y[0:4, :],
                                    ap(c0e_sb, [[1, 4]], off=4 * s),
                                    ap(x_sb[q], [[1, H]],
                                       off=s * 4 * H + ic * H),
                                    start=(s == 0 and ic == 0),
                                    stop=(s == 3 and ic == 3),
                                )
                        ysb = work.tile([4, H], BF16, tag="ysb0", bufs=4)
                        nc.scalar.activation(ysb[:], ps_y[0:4, :], AF.Copy)
                    else:
                        for s in range(QS):
                            for ic in range(4):
                                MM(
                                    ps_y[32 * s:32 * s + 32, :],
                                    ap(cT[q], [[1, O]], off=ic * P + s * O),
                                    ap(x_sb[q], [[1, H]],
                                       off=s * 4 * H + ic * H),
                                    start=(ic == 0),
                                    stop=(ic == 3),
                                    tile_position=(0, 32 * s),
                                )
                        ysb = work.tile([P, H], BF16, tag="ysb", bufs=4)
                        for hh in range(2):
                            nc.scalar.activation(
                                ysb[:, hh * P:(hh + 1) * P],
                                ps_y[:, hh * P:(hh + 1) * P], AF.Copy)
                    mark("act", ysb)
                    st["ysb"] = ysb

                def Bc():
                    CUR[0] = "Bc.q%d.t%d" % (q, t)
                    pe_sync("act")
                    ps_yt = ps.tile([P, 4 * P], BF16, tag="ytb")
                    if t == 0:
                        for hc in range(2):
                            T(ps_yt[:, hc * 4:(hc + 1) * 4],
                              st["ysb"][:4, hc * P:(hc + 1) * P],
                              id_sb[:4, :4])
                        ytsb = work.tile([P, 8], BF16, tag="ytsb0", bufs=4)
                        nc.scalar.activation(ytsb[:], ps_yt[:, :8], AF.Copy)
                    else:
                        for hc in range(2):
                            T(ps_yt[:, hc * P:(hc + 1) * P],
                              st["ysb"][:, hc * P:(hc + 1) * P], id_sb)
                        ytsb = work.tile([P, 2 * P], BF16, tag="ytsb", bufs=4)
                        nc.scalar.activation(ytsb[:], ps_yt[:, :2 * P],
                                             AF.Copy)
                    mark("act", ytsb)
                    st["ytsb"] = ytsb

                def C():
                    CUR[0] = "C.q%d.t%d" % (q, t)
                    pe_sync("dve")
                    if t == 0:
                        ps_vf = ps.tile([P, 16], F32, tag="vf",
                                        name="ps_vf0")
                        for g in range(4):
                            for hc in range(2):
                                MM(
                                    ps_vf[:, g * 4:(g + 1) * 4],
                                    ap(w_sb, [[1, P]], off=hc * OF + g * P),
                                    ap(st["ytsb"], [[1, 4]], off=hc * 4),
                                    start=(hc == 0),
                                    stop=(hc == 1),
                                )
                        vr = work.tile([P, 16], F32, tag="vr", bufs=4)
                        nc.vector.tensor_copy(vr[:], ps_vf[:])
                    else:
                        ps_vf = ps.tile([P, 4 * P], F32, tag="vf")
                        for g in range(4):
                            for hc in range(2):
                                MM(
                                    ps_vf[:, g * P:(g + 1) * P],
                                    ap(w_sb, [[1, P]], off=hc * OF + g * P),
                                    ap(st["ytsb"], [[1, P]], off=hc * P),
                                    start=(hc == 0),
                                    stop=(hc == 1),
                                )
                        msk = work.tile([P, 4 * P], BF16, tag="msk")
                        nc.vector.tensor_mul(
                            ap(msk, [[P, 4], [O, QS], [1, O]]),
                            ap(ps_vf, [[P, 4], [O, QS], [1, O]]),
                            ap(md_sb, [[O, 4], [0, QS], [1, O]]),
                        )
                        st["msk"] = msk
                        vr = work.tile([P, 16], F32, tag="vr", bufs=4)
                        nc.vector.reduce_sum(
                            out=vr[:],
                            in_=ap(msk, [[O, 16], [1, O]]),
                            axis=AX.X,
                        )
                    vrsq = work.tile([P, 16], BF16, tag="vrsq", bufs=4)
                    nc.vector.tensor_mul(vrsq[:], vr[:], vr[:])
                    mark("dve", vrsq)
                    st["vr"] = vr
                    st["vrsq"] = vrsq

                def D():
                    CUR[0] = "D.q%d.t%d" % (q, t)
                    pe_sync("dve")
                    mf = ps.tile([P, 32], F32, tag="mf")
                    if t > 0:
                        MM(mf[:8, :16], i16_sb,
                           ap(st["msk"], [[1, 16]]), label="ping")
                    MM(mf[:8, :16], i16_sb, st["vrsq"][:])
                    lnm = work.tile([8, 16], F32, tag="lnm", bufs=4)
                    nc.scalar.activation(lnm[:], mf[:8, :16], AF.Ln)
                    s0 = work.tile([8, 16], F32, tag="s0", bufs=4)
                    nc.scalar.activation(s0[:], lnm[:], AF.Exp, scale=0.5)
                    mark("act", s0)
                    onep = work.tile([8, 16], F32, tag="onep", bufs=4)
                    nc.vector.tensor_scalar_add(onep[:], mf[:8, :16], 1.0)
                    rp = work.tile([8, 16], F32, tag="rp", bufs=4)
                    nc.vector.reciprocal(rp[:], onep[:])
                    facb = work.tile([8, 16], BF16, tag="facb", bufs=4)
                    nc.vector.tensor_mul(facb[:], s0[:], rp[:])
                    mark("dve", facb)
                    st["mf"] = mf
                    st["facb"] = facb

                def E():
                    CUR[0] = "E.q%d.t%d" % (q, t)
                    mf = st["mf"]
                    pe_sync("dve", "act")
                    MM(mf[:, 16:32], e8_sb, st["facb"][:8, :])
                    if not last:
                        vsq = work.tile([P, 16], BF16, tag="vsq", bufs=4)
                        nc.vector.tensor_mul(vsq[:], st["vr"][:],
                                             mf[:, 16:32])
                        mark("dve", vsq)
                        vp = work.tile([P, 4 * P], BF16, tag="vp", bufs=4)
                        nc.vector.tensor_mul(
                            ap(vp, [[P, 4], [O, QS], [1, O]]),
                            ap(vsq, [[4, 4], [1, QS], [0, O]]),
                            ap(md_sb, [[O, 4], [0, QS], [1, O]]),
                        )
                        mark("pool", vp)
                        st["vp"] = vp
                    else:
                        # all quads write one [128, (q, s, g)] tile; single
                        # transpose + copy + cast-DMA at the last quad
                        if "vsq_all" not in fin:
                            fin["vsq_all"] = work.tile([P, 64], BF16,
                                                       tag="vsq_all", bufs=1,
                                                       name="vsq_all")
                        nc.vector.tensor_mul(
                            ap(fin["vsq_all"], [[1, 4], [4, 4]], off=16 * q),
                            ap(st["vr"], [[4, 4], [1, 4]]),
                            ap(mf, [[4, 4], [1, 4]], off=16),
                        )
                        fin["done"] = fin.get("done", 0) + 1
                        if fin["done"] == 2:
                            # first output half transposed early, hidden
                            # under the remaining quads' t2 compute
                            fin["ps_vo"] = ps.tile([P, P], BF16, tag="y",
                                                   name="ps_vo")
                            T(fin["ps_vo"][0:32, :P],
                              fin["vsq_all"][:, 0:32], id_sb, label="OUT:T1")
                        if fin["done"] == Q:
                            ps_vo = fin["ps_vo"]
                            T(ps_vo[32:64, :P], fin["vsq_all"][:, 32:64],
                              id_sb, label="OUT:T2")
                            # f32 cast in the DVE copy so the output DMA can
                            # use the sync engine's HW DGE (no gpsimd SW-DGE
                            # cast overhead on the tail)
                            vo = work.tile([64, P], F32, tag="vosb")
                            nc.vector.tensor_copy(vo[:], ps_vo[:64, :P])
                            nc.sync.dma_start(
                                out=dram_ap(out_d, [[P, 64], [1, P]]),
                                in_=vo[:],
                            )

                def Fc():
                    CUR[0] = "Fc.q%d.t%d" % (q, t)
                    pe_sync("dve")
                    ps_zt = ps.tile([P, 2 * P], F32, tag="zt")
                    for hc in range(2):
                        for m in range(4):
                            MM(
                                ps_zt[:, hc * P:(hc + 1) * P],
                                ap(wt_sb, [[1, P]], off=m * H + hc * P),
                                ap(st["vp"], [[1, P]], off=m * P),
                                start=(m == 0),
                                stop=(m == 3),
                            )
                    ztsb = work.tile([P, 2 * P], BF16, tag="ztsb", bufs=4)
                    for hh in range(2):
                        nc.scalar.activation(
                            ztsb[:, hh * P:(hh + 1) * P],
                            ps_zt[:, hh * P:(hh + 1) * P], AF.Copy)
                    mark("act", ztsb)
                    st["ztsb"] = ztsb

                def G():
                    CUR[0] = "G.q%d.t%d" % (q, t)
                    pe_sync("act", "xt%d" % q)
                    ps_b = ps.tile([P, I], F32, tag="b", bufs=2)
                    for s in range(QS):
                        for hc in range(2):
                            MM(
                                ps_b[32 * s:32 * s + 32, :],
                                ap(st["ztsb"], [[1, O]], off=hc * P + s * O),
                                ap(xt_sb[q], [[1, I]], off=s * 2 * I + hc * I),
                                start=(hc == 0),
                                stop=(hc == 1),
                                tile_position=(0, 32 * s),
                            )
                    eb = work.tile([P, I], BF16, tag="eb", bufs=4)
                    for hh in range(2):
                        nc.scalar.activation(
                            eb[:, hh * 2 * P:(hh + 1) * 2 * P],
                            ps_b[:, hh * 2 * P:(hh + 1) * 2 * P], AF.Exp)
                    mark("act", eb)
                    st["eb"] = eb

                def Hc():
                    CUR[0] = "Hc.q%d.t%d" % (q, t)
                    pe_sync("act")
                    ps_ebt = ps.tile([P, 4 * P], BF16, tag="ebt")
                    for ic in range(4):
                        T(ps_ebt[:, ic * P:(ic + 1) * P],
                          st["eb"][:, ic * P:(ic + 1) * P], id_sb)
                    ssum = work.tile([P, 16], F32, tag="ssum")
                    nc.vector.reduce_sum(
                        out=ssum[:],
                        in_=ap(ps_ebt, [[O, 16], [1, O]]),
                        axis=AX.X,
                    )
                    rs = work.tile([P, 16], F32, tag="rs")
                    nc.vector.reciprocal(rs[:], ssum[:])
                    ct = work.tile([P, 4 * P], BF16, tag="ct%d" % q, bufs=1)
                    nc.vector.tensor_mul(
                        ap(ct, [[O, 16], [1, O]]),
                        ap(ps_ebt, [[O, 16], [1, O]]),
                        ap(rs, [[1, 16], [0, O]]),
                    )
                    mark("dve", ct)
                    cT[q] = ct

                if last:
                    return [A, Bc, C, D, E]
                return [A, Bc, C, D, E, Fc, G, Hc]

            all_chunks = {q: mk_chunks(q) for q in range(Q)}
            L = len(all_chunks[0])
            SKEW = 2
            for k in range(L + SKEW * (Q - 1)):
                for q in reversed(range(Q)):
                    c = k - SKEW * q
                    if 0 <= c < L:
                        all_chunks[q][c]()

    if split_waits:
        _split_fat_waits(nc)
    return nc


def _split_fat_waits(nc, maxw=1):
    """Walrus caps sync waits per instruction; split overflow onto extra
    same-engine Drain instructions inserted just before the offender."""
    nsplit = 0
    for blk in nc.m.functions[0].blocks:
        new_insts = []
        for inst in blk.instructions:
            si = getattr(inst, "sync_info", None)
            w = list(si.on_wait) if si is not None and si.on_wait else []
            if len(w) > maxw:
                for k in range(0, len(w) - maxw, maxw):
                    d = mybir.InstDrain(name="I-waitsplit-%d" % nsplit,
                                        ins=[], outs=[])
                    nsplit += 1
                    d.engine = inst.engine
                    d.sync_info = mybir.SyncInfo(on_wait=w[k:k + maxw],
                                                 on_update=[])
                    new_insts.append(d)
                si.on_wait = w[len(w) - maxw:]
            new_insts.append(inst)
        blk.instructions[:] = new_insts
    return nc


_NC_CACHE = None


def make_cst(Wn):
    """bf16 constant blob [128, CSTN] matching the device-side layout."""
    cst = np.zeros((P, CSTN), np.float32)
    cst[:, CW:CW + 2 * OF] = (
        Wn.reshape(2, P, OF).transpose(1, 0, 2).reshape(P, 2 * OF))
    cst[:, CWT:CWT + 4 * H] = (
        Wn.T.reshape(4, P, H).transpose(1, 0, 2).reshape(P, 4 * H))
    cst[:, CID:CID + P] = np.eye(P, dtype=np.float32)
    for p in range(P):
        for g in range(4):
            cst[p, CMD + g * O + g * 8 + p // 16] = 1.0
    cst[np.arange(P), CI16 + np.arange(P) // 16] = 1.0
    for j in range(8):
        cst[j, CE8 + 16 * j:CE8 + 16 * (j + 1)] = 1.0
    cst[:, CC0:CC0 + O] = 1.0 / O
    for s in range(4):
        cst[:, CC0E + 4 * s + s] = 1.0 / O
    out = cst.astype(ml_dtypes.bfloat16)
    # bf16 pair (0.0, 1.0) little-endian == f32 1.0 when viewed 4-byte
    out[:, CIDF] = ml_dtypes.bfloat16(0.0)
    out[:, CIDF + 1] = ml_dtypes.bfloat16(1.0)
    return out


def make_in_maps(x, W):
    x = np.asarray(x, dtype=np.float32)
    Wn = np.asarray(W, dtype=np.float32).reshape(H, OF)
    cst = make_cst(Wn)
    xq = x.astype(ml_dtypes.bfloat16).reshape(NCORES, Q, QS, 4, P, H)
    # xb[c, q, p, (s, ic, h)] = x[c, 4q+s, 128ic+p, h]
    xb = np.ascontiguousarray(xq.transpose(0, 1, 4, 2, 3, 5)).reshape(
        NCORES, Q, P, QS * 4 * H)
    # xtb[c, q, p, (s, hc, i)] = x[c, 4q+s, i, 128hc+p]
    xth = x.astype(ml_dtypes.bfloat16).reshape(NCORES, Q, QS, I, 2, P)
    xtb = np.ascontiguousarray(xth.transpose(0, 1, 5, 2, 4, 3)).reshape(
        NCORES, Q, P, QS * 2 * I)
    return [
        {"xb": xb[c], "xtb": xtb[c], "cst": cst}
        for c in range(NCORES)
    ]


def kernel(x: np.ndarray, W: np.ndarray) -> np.ndarray:
    global _NC_CACHE
    if _NC_CACHE is None:
        _NC_CACHE = build_program()
    in_maps = make_in_maps(x, W)
    res = run_bass_kernel_spmd(_NC_CACHE, in_maps, core_ids=list(range(NCORES)))
    out = np.stack([res.results[c]["out"] for c in range(NCORES)])
    return out.reshape(B, O, F)



# revision 16
# speedup vs baseline: 1.1059x; 1.0252x over previous
"""Trainium2 Bass kernel for nn_Capsule (dynamic routing), bf16 dataflow.

reference: u = x @ W  (per-sample [512,256]@[256,512]); b=0
           3x { c = softmax_o(b); v[o,f] = sum_i c[o,i] u[i,(o,f)];
                v = squash(v); b[o,i] = sum_f v[o,f] u[i,(o,f)] }
           return v [B, 32, 16]

u is never materialized.  Per core: 16 samples = 4 quads of 4.
All matmuls in bf16 (1 cyc/row, fast LDWEIGHTS); fp32 only in PSUM and
the squash scalar chain.  Host ships x twice (natural + pre-transposed)
in bf16, so no on-device transposes of x are needed.

Per (iter t, quad q), layouts ([partition, free]):
  cT   [i%128, (ic4, s4, o32)]  bf16   (t=0: uniform 1/32)
  y    = cT.T @ x          -> ps_y  [(s,o)128, h256]      (16 MM ap256)
  yT   via 2 PE transposes -> yt_sb [h%128, (hc2, so128)] bf16
  vfT  = W @ yT            -> ps_vf [of%128, (g4, so128)] (8 MM ap128)
  diag: mask-mul (gpsimd) + free-reduce over o' (DVE) -> vr [of%128,(g4,s4)] f32
  mag  = I16.T @ vr^2 (PE) -> [o-sub 8, (g,s)16]; factor = exp(.5 ln m)/(1+m)
         (Ln+Exp share one act-table set -> no table reloads)
  fac128 = E8.T @ factor (PE broadcast over f partitions)
  vsq  = vr * fac128 -> bf16
  VmatT: vsq bcast over o' * mask (gpsimd) -> vp [of%128, (g4,s4,o'32)] bf16
  zT   = WT @ VmatT        -> ps_zt [h%128, (hc2, so128)] (8 MM ap128)
  b    = zt.T @ xT         -> ps_b  [(s,o)128, i512]      (8 MM ap512)
  eb   = exp(b) (Act, bf16); ebT via 4 PE transposes (bf16 PSUM)
  softmax over o in [i, (ic,s,o)] layout -> cT for next iter
Last iter stops after vsq; output transposed on PE and cast bf16->f32
by a gpsimd DMA.
"""

import numpy as np
import ml_dtypes

import concourse.bass as bass
import concourse.tile as tile
from concourse import mybir
from concourse.bass_utils import run_bass_kernel_spmd

F32 = mybir.dt.float32
BF16 = mybir.dt.bfloat16
F8 = mybir.dt.float8e4
ZSC = 16.0            # z prescale before fp8 quant (undone in eb's exp)
AF = mybir.ActivationFunctionType
AX = mybir.AxisListType

B, I, H = 128, 512, 256
O, F = 32, 16
OF = O * F            # 512
NCORES = 8
S = B // NCORES       # 16 samples per core
Q = 4                 # quads per core
QS = 4                # samples per quad
NIT = 3
P = 128

# bf16 constant blob [128, CSTN]
CW = 0                # W  [h%128, (hc2, of512)]
CWT = CW + 2 * OF     # WT [of%128, (m4, h256)]
CID = CWT + 4 * H     # identity [128, 128]
CMD = CID + P         # diag mask [128, (g4, o32)]: md[p,(g,o)] = (o == g*8+p//16)
CI16 = CMD + 4 * O    # [128, 8]: i16[p, j] = (j == p//16)
CE8 = CI16 + 8        # [8, 128]: e8[j, p] = (p//16 == j)
CC0 = CE8 + P         # [128, 32] = 1/32
CIDF = CC0 + O        # 2 bf16 cols whose bytes alias to f32 1.0
CC0E = CIDF + 2       # [128, 16]: c0e[i, 4s+j] = (j==s)/32  (t0 A lhsT)
CSTN = CC0E + 16


def ap(t, dims, off=0):
    """AP over tile/handle `t`: keep partition dim, explicit free dims."""
    a = t if isinstance(t, bass.AP) else t[:]
    return bass.AP(tensor=a.tensor, offset=a.offset + off,
                   ap=[list(a.ap[0])] + [list(d) for d in dims])


def f32view(a, off=0):
    """1x1 f32 alias of an SBUF AP's base (for sync-anchor reads only)."""
    t = a.tensor
    t2 = t if t.dtype == F32 else bass.SBTensorHandle(
        name=t.name, shape=[t.shape[0], t.shape[1] // 2], dtype=F32,
        base_partition=t.base_partition,
        manual_sbuf_range=t.manual_sbuf_range,
        manual_base_name=t.manual_base_name)
    return bass.AP(tensor=t2, offset=off,
                   ap=[[int(t2.shape[1]), 1], [1, 1]])


def dram_ap(handle, dims, off=0):
    """AP over DRAM handle with fully explicit dims (first = partition)."""
    a = handle[:]
    return bass.AP(tensor=a.tensor, offset=a.offset + off,
                   ap=[list(d) for d in dims])


MM_LABELS = []


def build_program(split_waits=True):
    MM_LABELS.clear()
    nc = bass.Bass("TRN2", target_bir_lowering=False)

    x_d = nc.dram_tensor("xb", [Q, P, QS * 4 * H], BF16, kind="ExternalInput")
    xt_d = nc.dram_tensor("xtb", [Q, P, QS * 2 * I], BF16, kind="ExternalInput")
    cst_d = nc.dram_tensor("cst", [P, CSTN], BF16, kind="ExternalInput")
    out_d = nc.dram_tensor("out", [S, OF], F32, kind="ExternalOutput")

    with tile.TileContext(nc) as tc:
        with (
            tc.tile_pool(name="consts", bufs=1) as consts,
            tc.tile_pool(name="xpool", bufs=4) as xpool,
            tc.tile_pool(name="xtpool", bufs=4) as xtpool,
            tc.tile_pool(name="work", bufs=2) as work,
            tc.tile_pool(name="ps", bufs=1, space="PSUM") as ps,
        ):
            cst = consts.tile([P, CSTN], BF16)

            def load_cst(c0, c1):
                nc.sync.dma_start(
                    out=ap(cst, [[1, c1 - c0]], off=c0),
                    in_=dram_ap(cst_d, [[CSTN, P], [1, c1 - c0]], off=c0),
                )
            w_sb = cst[:, CW:CW + 2 * OF]
            wt_sb = cst[:, CWT:CWT + 4 * H]
            id_sb = cst[:, CID:CID + P]
            md_sb = cst[:, CMD:CMD + 4 * O]
            i16_sb = cst[:, CI16:CI16 + 8]
            e8_sb = cst[:8, CE8:CE8 + P]
            c0_sb = cst[:, CC0:CC0 + O]
            c0e_sb = cst[:, CC0E:CC0E + 16]

            # PE sync anchors: PE observes foreign engine clocks via 1x1
            # transposes so walrus can elide per-instruction waits.
            anch = None  # anchors disabled; bank freed for the vo tile
            idf1 = f32view(cst[:], off=CIDF // 2)
            dirty = {}
            acol = [0]
            pending = []

            def mark(key, apv):
                dirty[key] = apv

            def pe_sync(*keys, force=False):
                # Anchors proved to over-serialize (PE waits on the globally
                # newest op of an engine, not the actual dependency); rely on
                # Tile's precise per-instruction deps instead.
                pending.clear()
                if not force:
                    return
                for k in keys:
                    if k not in dirty:
                        continue
                    d = dirty.pop(k)
                    MM_LABELS.append("anch:" + k)
                    a = nc.tensor.transpose(
                        anch[:1, acol[0]:acol[0] + 1], f32view(d),
                        idf1)
                    pending.append(a.ins)
                    acol[0] = (acol[0] + 1) % 16

            def _chain(b):
                for a in pending:
                    bass._add_dep_helper(b.ins, a, sync=False,
                                         reason="pe-anchor order")
                return b

            def T(out, in_, ident, label="", **kw):
                MM_LABELS.append(label or CUR[0] + ":T")
                return _chain(nc.tensor.transpose(out, in_, ident, **kw))

            def MM(out, lhsT, rhs, label="", **kw):
                MM_LABELS.append(label or CUR[0])
                return _chain(nc.tensor.matmul(out, lhsT, rhs, **kw))

            mark("cst", cst)
            CUR = ["init"]

            # ---- input DMAs (x natural + pre-transposed, interleaved so
            # quad q's xT lands before its first b-stage) ----
            x_sb = {}
            xt_sb = {}

            NXW = QS * 4 * H

            def load_x(q, halves=1):
                xs = xpool.tile([P, NXW], BF16, tag="x", name="xq%d" % q)
                hw_ = NXW // halves
                for h in range(halves):
                    nc.sync.dma_start(
                        out=ap(xs, [[1, hw_]], off=h * hw_),
                        in_=dram_ap(x_d, [[NXW, P], [1, hw_]],
                                    off=q * P * NXW + h * hw_),
                    )
                mark("x%d" % q, xs)
                x_sb[q] = xs

            def load_xt(q):
                xt = xtpool.tile([P, NXW], BF16, tag="xt")
                nc.sync.dma_start(
                    out=ap(xt, [[1, NXW]]),
                    in_=dram_ap(xt_d, [[NXW, P], [1, NXW]], off=q * P * NXW),
                )
                mark("xt%d" % q, xt)
                xt_sb[q] = xt

            # JIT order: small consts, x0, W, x1, xt0, x2, xt1, x3, xt2, xt3
            load_cst(CID, CSTN)
            load_x(0)
            load_cst(CW, CID)
            load_x(1)
            load_xt(0)
            load_x(2)
            load_xt(1)
            load_x(3)
            load_xt(2)
            load_xt(3)

            # PE p-state warmup: ~14 dummy matmuls on early consts while
            # the x0 DMA streams.  The PE clock ramps to full speed after
            # ~3us of continuous execution (pstate low/mid otherwise), so
            # burning the DMA-wait keeps the real A-stage at full clock.
            ps_w = ps.tile([P, H], F32, tag="y", name="ps_warm")
            for _wi in range(16):
                MM(ps_w[:, :H], id_sb,
                   ap(cst, [[1, H]], off=CID), label="warm")

            cT = {q: None for q in range(Q)}
            fin = {}

            def mk_chunks(q):
                """Per-quad list of emission chunks; wavefront-interleaved
                across quads so PE always has another quad's work during
                cross-engine chain latencies."""
                st = {}
                chunks = []
                for t in range(NIT):
                    chunks.extend(stage_fns(q, t, st))
                return chunks

            def stage_fns(q, t, st):
                last = (t == NIT - 1)

                def A():
                    CUR[0] = "A.q%d.t%d" % (q, t)
                    pe_sync("cst", "x%d" % q, "dve")
                    ps_y = ps.tile([P, H], F32, tag="y")
                    if t == 0:
                        # uniform c: y0 is o-independent -> [4, 256] rows,
                        # one 16-MM accumulation group via c0e selectors
                        for s in range(QS):
                            for ic in range(4):
                                MM(
                                    ps_y[0:4, :],
                                    ap(c0e_sb, [[1, 4]], off=4 * s),
                                    ap(x_sb[q], [[1, H]],
                                       off=s * 4 * H + ic * H),
                                    start=(s == 0 and ic == 0),
                                    stop=(s == 3 and ic == 3),
                                )
                        ysb = work.tile([4, H], BF16, tag="ysb0", bufs=4)
                        nc.scalar.activation(ysb[:], ps_y[0:4, :], AF.Copy)
                    else:
                        for s in range(QS):
                            for ic in range(4):
                                MM(
                                    ps_y[32 * s:32 * s + 32, :],
                                    ap(cT[q], [[1, O]], off=ic * P + s * O),
                                    ap(x_sb[q], [[1, H]],
                                       off=s * 4 * H + ic * H),
                                    start=(ic == 0),
                                    stop=(ic == 3),
                                    tile_position=(0, 32 * s),
                                )
                        ysb = work.tile([P, H], BF16, tag="ysb", bufs=4)
                        for hh in range(2):
                            nc.scalar.activation(
                                ysb[:, hh * P:(hh + 1) * P],
                                ps_y[:, hh * P:(hh + 1) * P], AF.Copy)
                    mark("act", ysb)
                    st["ysb"] = ysb

                def Bc():
                    CUR[0] = "Bc.q%d.t%d" % (q, t)
                    pe_sync("act")
                    ps_yt = ps.tile([P, 4 * P], BF16, tag="ytb")
                    if t == 0:
                        for hc in range(2):
                            T(ps_yt[:, hc * 4:(hc + 1) * 4],
                              st["ysb"][:4, hc * P:(hc + 1) * P],
                              id_sb[:4, :4])
                        ytsb = work.tile([P, 8], BF16, tag="ytsb0", bufs=4)
                        nc.scalar.activation(ytsb[:], ps_yt[:, :8], AF.Copy)
                    else:
                        for hc in range(2):
                            T(ps_yt[:, hc * P:(hc + 1) * P],
                              st["ysb"][:, hc * P:(hc + 1) * P], id_sb)
                        ytsb = work.tile([P, 2 * P], BF16, tag="ytsb", bufs=4)
                        nc.vector.tensor_copy(ytsb[:], ps_yt[:, :2 * P])
                    mark("act", ytsb)
                    st["ytsb"] = ytsb

                def C():
                    CUR[0] = "C.q%d.t%d" % (q, t)
                    pe_sync("dve")
                    if t == 0:
                        ps_vf = ps.tile([P, 16], F32, tag="vf",
                                        name="ps_vf0")
                        for g in range(4):
                            for hc in range(2):
                                MM(
                                    ps_vf[:, g * 4:(g + 1) * 4],
                                    ap(w_sb, [[1, P]], off=hc * OF + g * P),
                                    ap(st["ytsb"], [[1, 4]], off=hc * 4),
                                    start=(hc == 0),
                                    stop=(hc == 1),
                                )
                        vr = work.tile([P, 16], F32, tag="vr", bufs=4)
                        nc.vector.tensor_copy(vr[:], ps_vf[:])
                    else:
                        ps_vf = ps.tile([P, 4 * P], F32, tag="vf")
                        for g in range(4):
                            for hc in range(2):
                                MM(
                                    ps_vf[:, g * P:(g + 1) * P],
                                    ap(w_sb, [[1, P]], off=hc * OF + g * P),
                                    ap(st["ytsb"], [[1, P]], off=hc * P),
                                    start=(hc == 0),
                                    stop=(hc == 1),
                                )
                        msk = work.tile([P, 4 * P], BF16, tag="msk")
                        nc.vector.tensor_mul(
                            ap(msk, [[P, 4], [O, QS], [1, O]]),
                            ap(ps_vf, [[P, 4], [O, QS], [1, O]]),
                            ap(md_sb, [[O, 4], [0, QS], [1, O]]),
                        )
                        st["msk"] = msk
                        vr = work.tile([P, 16], F32, tag="vr", bufs=4)
                        nc.vector.reduce_sum(
                            out=vr[:],
                            in_=ap(msk, [[O, 16], [1, O]]),
                            axis=AX.X,
                        )
                    vrsq = work.tile([P, 16], BF16, tag="vrsq", bufs=4)
                    nc.vector.tensor_mul(vrsq[:], vr[:], vr[:])
                    mark("dve", vrsq)
                    st["vr"] = vr
                    st["vrsq"] = vrsq

                def D():
                    CUR[0] = "D.q%d.t%d" % (q, t)
                    pe_sync("dve")
                    mf = ps.tile([P, 32], F32, tag="mf")
                    if t > 0:
                        MM(mf[:8, :16], i16_sb,
                           ap(st["msk"], [[1, 16]]), label="ping")
                    MM(mf[:8, :16], i16_sb, st["vrsq"][:])
                    lnm = work.tile([8, 16], F32, tag="lnm", bufs=4)
                    nc.scalar.activation(lnm[:], mf[:8, :16], AF.Ln)
                    s0 = work.tile([8, 16], F32, tag="s0", bufs=4)
                    nc.scalar.activation(s0[:], lnm[:], AF.Exp, scale=0.5)
                    mark("act", s0)
                    onep = work.tile([8, 16], F32, tag="onep", bufs=4)
                    nc.vector.tensor_scalar_add(onep[:], mf[:8, :16], 1.0)
                    rp = work.tile([8, 16], F32, tag="rp", bufs=4)
                    nc.vector.reciprocal(rp[:], onep[:])
                    facb = work.tile([8, 16], BF16, tag="facb", bufs=4)
                    nc.vector.tensor_mul(facb[:], s0[:], rp[:])
                    mark("dve", facb)
                    st["mf"] = mf
                    st["facb"] = facb

                def E():
                    CUR[0] = "E.q%d.t%d" % (q, t)
                    mf = st["mf"]
                    pe_sync("dve", "act")
                    MM(mf[:, 16:32], e8_sb, st["facb"][:8, :])
                    if not last:
                        vsq = work.tile([P, 16], BF16, tag="vsq", bufs=4)
                        nc.vector.tensor_mul(vsq[:], st["vr"][:],
                                             mf[:, 16:32])
                        mark("dve", vsq)
                        vp = work.tile([P, 4 * P], BF16, tag="vp", bufs=4)
                        nc.vector.tensor_mul(
                            ap(vp, [[P, 4], [O, QS], [1, O]]),
                            ap(vsq, [[4, 4], [1, QS], [0, O]]),
                            ap(md_sb, [[O, 4], [0, QS], [1, O]]),
                        )
                        mark("pool", vp)
                        st["vp"] = vp
                    else:
                        # all quads write one [128, (q, s, g)] tile; single
                        # transpose + copy + cast-DMA at the last quad
                        if "vsq_all" not in fin:
                            fin["vsq_all"] = work.tile([P, 64], BF16,
                                                       tag="vsq_all", bufs=1,
                                                       name="vsq_all")
                        nc.vector.tensor_mul(
                            ap(fin["vsq_all"], [[1, 4], [4, 4]], off=16 * q),
                            ap(st["vr"], [[4, 4], [1, 4]]),
                            ap(mf, [[4, 4], [1, 4]], off=16),
                        )
                        fin["done"] = fin.get("done", 0) + 1
                        if fin["done"] == 2:
                            # first output half transposed early, hidden
                            # under the remaining quads' t2 compute
                            fin["ps_vo"] = ps.tile([P, P], BF16, tag="y",
                                                   name="ps_vo")
                            T(fin["ps_vo"][0:32, :P],
                              fin["vsq_all"][:, 0:32], id_sb, label="OUT:T1")
                        if fin["done"] == Q:
                            ps_vo = fin["ps_vo"]
                            T(ps_vo[32:64, :P], fin["vsq_all"][:, 32:64],
                              id_sb, label="OUT:T2")
                            # f32 cast in the DVE copy so the output DMA can
                            # use the sync engine's HW DGE (no gpsimd SW-DGE
                            # cast overhead on the tail)
                            vo = work.tile([64, P], F32, tag="vosb")
                            nc.vector.tensor_copy(vo[:], ps_vo[:64, :P])
                            nc.sync.dma_start(
                                out=dram_ap(out_d, [[P, 64], [1, P]]),
                                in_=vo[:],
                            )

                def Fc():
                    CUR[0] = "Fc.q%d.t%d" % (q, t)
                    pe_sync("dve")
                    ps_zt = ps.tile([P, 2 * P], F32, tag="zt")
                    for hc in range(2):
                        for m in range(4):
                            MM(
                                ps_zt[:, hc * P:(hc + 1) * P],
                                ap(wt_sb, [[1, P]], off=m * H + hc * P),
                                ap(st["vp"], [[1, P]], off=m * P),
                                start=(m == 0),
                                stop=(m == 3),
                            )
                    ztsb = work.tile([P, 2 * P], BF16, tag="ztsb", bufs=4)
                    for hh in range(2):
                        nc.scalar.activation(
                            ztsb[:, hh * P:(hh + 1) * P],
                            ps_zt[:, hh * P:(hh + 1) * P], AF.Copy)
                    mark("act", ztsb)
                    st["ztsb"] = ztsb

                def G():
                    CUR[0] = "G.q%d.t%d" % (q, t)
                    pe_sync("act", "xt%d" % q)
                    ps_b = ps.tile([P, I], F32, tag="b", bufs=2)
                    for s in range(QS):
                        for hc in range(2):
                            MM(
                                ps_b[32 * s:32 * s + 32, :],
                                ap(st["ztsb"], [[1, O]], off=hc * P + s * O),
                                ap(xt_sb[q], [[1, I]], off=s * 2 * I + hc * I),
                                start=(hc == 0),
                                stop=(hc == 1),
                                tile_position=(0, 32 * s),
                            )
                    eb = work.tile([P, I], BF16, tag="eb", bufs=4)
                    for hh in range(2):
                        nc.scalar.activation(
                            eb[:, hh * 2 * P:(hh + 1) * 2 * P],
                            ps_b[:, hh * 2 * P:(hh + 1) * 2 * P], AF.Exp)
                    mark("act", eb)
                    st["eb"] = eb

                def Hc():
                    CUR[0] = "Hc.q%d.t%d" % (q, t)
                    pe_sync("act")
                    ps_ebt = ps.tile([P, 4 * P], BF16, tag="ebt")
                    for ic in range(4):
                        T(ps_ebt[:, ic * P:(ic + 1) * P],
                          st["eb"][:, ic * P:(ic + 1) * P], id_sb)
                    ssum = work.tile([P, 16], F32, tag="ssum")
                    nc.vector.reduce_sum(
                        out=ssum[:],
                        in_=ap(ps_ebt, [[O, 16], [1, O]]),
                        axis=AX.X,
                    )
                    rs = work.tile([P, 16], F32, tag="rs")
                    nc.vector.reciprocal(rs[:], ssum[:])
                    ct = work.tile([P, 4 * P], BF16, tag="ct%d" % q, bufs=1)
                    nc.vector.tensor_mul(
                        ap(ct, [[O, 16], [1, O]]),
                        ap(ps_ebt, [[O, 16], [1, O]]),
                        ap(rs, [[1, 16], [0, O]]),
                    )
                    mark("dve", ct)
                    cT[q] = ct

                if last:
                    return [A, Bc, C, D, E]
                return [A, Bc, C, D, E, Fc, G, Hc]

            all_chunks = {q: mk_chunks(q) for q in range(Q)}
            L = len(all_chunks[0])
            SKEW = 2
            for k in range(L + SKEW * (Q - 1)):
                for q in reversed(range(Q)):
                    c = k - SKEW * q
                    if 0 <= c < L:
                        all_chunks[q][c]()

    if split_waits:
        _split_fat_waits(nc)
    return nc


def _split_fat_waits(nc, maxw=1):
    """Walrus caps sync waits per instruction; split overflow onto extra
    same-engine Drain instructions inserted just before the offender."""
    nsplit = 0
    for blk in nc.m.functions[0].blocks:
        new_insts = []
        for inst in blk.instructions:
            si = getattr(inst, "sync_info", None)
            w = list(si.on_wait) if si is not None and si.on_wait else []
            if len(w) > maxw:
                for k in range(0, len(w) - maxw, maxw):
                    d = mybir.InstDrain(name="I-waitsplit-%d" % nsplit,
                                        ins=[], outs=[])
                    nsplit += 1
                    d.engine = inst.engine
                    d.sync_info = mybir.SyncInfo(on_wait=w[k:k + maxw],
                                                 on_update=[])
                    new_insts.append(d)
                si.on_wait = w[len(w) - maxw:]
            new_insts.append(inst)
        blk.instructions[:] = new_insts
    return nc


_NC_CACHE = None


def make_cst(Wn):
    """bf16 constant blob [128, CSTN] matching the device-side layout."""
    cst = np.zeros((P, CSTN), np.float32)
    cst[:, CW:CW + 2 * OF] = (
        Wn.reshape(2, P, OF).transpose(1, 0, 2).reshape(P, 2 * OF))
    cst[:, CWT:CWT + 4 * H] = (
        Wn.T.reshape(4, P, H).transpose(1, 0, 2).reshape(P, 4 * H))
    cst[:, CID:CID + P] = np.eye(P, dtype=np.float32)
    for p in range(P):
        for g in range(4):
            cst[p, CMD + g * O + g * 8 + p // 16] = 1.0
    cst[np.arange(P), CI16 + np.arange(P) // 16] = 1.0
    for j in range(8):
        cst[j, CE8 + 16 * j:CE8 + 16 * (j + 1)] = 1.0
    cst[:, CC0:CC0 + O] = 1.0 / O
    for s in range(4):
        cst[:, CC0E + 4 * s + s] = 1.0 / O
    out = cst.astype(ml_dtypes.bfloat16)
    # bf16 pair (0.0, 1.0) little-endian == f32 1.0 when viewed 4-byte
    out[:, CIDF] = ml_dtypes.bfloat16(0.0)
    out[:, CIDF + 1] = ml_dtypes.bfloat16(1.0)
    return out


def make_in_maps(x, W):
    x = np.asarray(x, dtype=np.float32)
    Wn = np.asarray(W, dtype=np.float32).reshape(H, OF)
    cst = make_cst(Wn)
    xq = x.astype(ml_dtypes.bfloat16).reshape(NCORES, Q, QS, 4, P, H)
    # xb[c, q, p, (s, ic, h)] = x[c, 4q+s, 128ic+p, h]
    xb = np.ascontiguousarray(xq.transpose(0, 1, 4, 2, 3, 5)).reshape(
        NCORES, Q, P, QS * 4 * H)
    # xtb[c, q, p, (s, hc, i)] = x[c, 4q+s, i, 128hc+p]
    xth = x.astype(ml_dtypes.bfloat16).reshape(NCORES, Q, QS, I, 2, P)
    xtb = np.ascontiguousarray(xth.transpose(0, 1, 5, 2, 4, 3)).reshape(
        NCORES, Q, P, QS * 2 * I)
    return [
        {"xb": xb[c], "xtb": xtb[c], "cst": cst}
        for c in range(NCORES)
    ]


def kernel(x: np.ndarray, W: np.ndarray) -> np.ndarray:
    global _NC_CACHE
    if _NC_CACHE is None:
        _NC_CACHE = build_program()
    in_maps = make_in_maps(x, W)
    res = run_bass_kernel_spmd(_NC_CACHE, in_maps, core_ids=list(range(NCORES)))
    out = np.stack([res.results[c]["out"] for c in range(NCORES)])
    return out.reshape(B, O, F)



# revision 17
# speedup vs baseline: 1.1137x; 1.0071x over previous
"""Trainium2 Bass kernel for nn_Capsule (dynamic routing), bf16 dataflow.

reference: u = x @ W  (per-sample [512,256]@[256,512]); b=0
           3x { c = softmax_o(b); v[o,f] = sum_i c[o,i] u[i,(o,f)];
                v = squash(v); b[o,i] = sum_f v[o,f] u[i,(o,f)] }
           return v [B, 32, 16]

u is never materialized.  Per core: 16 samples = 4 quads of 4.
All matmuls in bf16 (1 cyc/row, fast LDWEIGHTS); fp32 only in PSUM and
the squash scalar chain.  Host ships x twice (natural + pre-transposed)
in bf16, so no on-device transposes of x are needed.

Per (iter t, quad q), layouts ([partition, free]):
  cT   [i%128, (ic4, s4, o32)]  bf16   (t=0: uniform 1/32)
  y    = cT.T @ x          -> ps_y  [(s,o)128, h256]      (16 MM ap256)
  yT   via 2 PE transposes -> yt_sb [h%128, (hc2, so128)] bf16
  vfT  = W @ yT            -> ps_vf [of%128, (g4, so128)] (8 MM ap128)
  diag: mask-mul (gpsimd) + free-reduce over o' (DVE) -> vr [of%128,(g4,s4)] f32
  mag  = I16.T @ vr^2 (PE) -> [o-sub 8, (g,s)16]; factor = exp(.5 ln m)/(1+m)
         (Ln+Exp share one act-table set -> no table reloads)
  fac128 = E8.T @ factor (PE broadcast over f partitions)
  vsq  = vr * fac128 -> bf16
  VmatT: vsq bcast over o' * mask (gpsimd) -> vp [of%128, (g4,s4,o'32)] bf16
  zT   = WT @ VmatT        -> ps_zt [h%128, (hc2, so128)] (8 MM ap128)
  b    = zt.T @ xT         -> ps_b  [(s,o)128, i512]      (8 MM ap512)
  eb   = exp(b) (Act, bf16); ebT via 4 PE transposes (bf16 PSUM)
  softmax over o in [i, (ic,s,o)] layout -> cT for next iter
Last iter stops after vsq; output transposed on PE and cast bf16->f32
by a gpsimd DMA.
"""

import numpy as np
import ml_dtypes

import concourse.bass as bass
import concourse.tile as tile
from concourse import mybir
from concourse.bass_utils import run_bass_kernel_spmd

F32 = mybir.dt.float32
BF16 = mybir.dt.bfloat16
F8 = mybir.dt.float8e4
ZSC = 16.0            # z prescale before fp8 quant (undone in eb's exp)
AF = mybir.ActivationFunctionType
AX = mybir.AxisListType

B, I, H = 128, 512, 256
O, F = 32, 16
OF = O * F            # 512
NCORES = 8
S = B // NCORES       # 16 samples per core
Q = 4                 # quads per core
QS = 4                # samples per quad
NIT = 3
P = 128

# bf16 constant blob [128, CSTN]
CW = 0                # W  [h%128, (hc2, of512)]
CWT = CW + 2 * OF     # WT [of%128, (m4, h256)]
CID = CWT + 4 * H     # identity [128, 128]
CMD = CID + P         # diag mask [128, (g4, o32)]: md[p,(g,o)] = (o == g*8+p//16)
CI16 = CMD + 4 * O    # [128, 8]: i16[p, j] = (j == p//16)
CE8 = CI16 + 8        # [8, 128]: e8[j, p] = (p//16 == j)
CC0 = CE8 + P         # [128, 32] = 1/32
CIDF = CC0 + O        # 2 bf16 cols whose bytes alias to f32 1.0
CC0E = CIDF + 2       # [128, 16]: c0e[i, 4s+j] = (j==s)/32  (t0 A lhsT)
CSTN = CC0E + 16


def ap(t, dims, off=0):
    """AP over tile/handle `t`: keep partition dim, explicit free dims."""
    a = t if isinstance(t, bass.AP) else t[:]
    return bass.AP(tensor=a.tensor, offset=a.offset + off,
                   ap=[list(a.ap[0])] + [list(d) for d in dims])


def f32view(a, off=0):
    """1x1 f32 alias of an SBUF AP's base (for sync-anchor reads only)."""
    t = a.tensor
    t2 = t if t.dtype == F32 else bass.SBTensorHandle(
        name=t.name, shape=[t.shape[0], t.shape[1] // 2], dtype=F32,
        base_partition=t.base_partition,
        manual_sbuf_range=t.manual_sbuf_range,
        manual_base_name=t.manual_base_name)
    return bass.AP(tensor=t2, offset=off,
                   ap=[[int(t2.shape[1]), 1], [1, 1]])


def dram_ap(handle, dims, off=0):
    """AP over DRAM handle with fully explicit dims (first = partition)."""
    a = handle[:]
    return bass.AP(tensor=a.tensor, offset=a.offset + off,
                   ap=[list(d) for d in dims])


MM_LABELS = []


def build_program(split_waits=True):
    MM_LABELS.clear()
    nc = bass.Bass("TRN2", target_bir_lowering=False)

    x_d = nc.dram_tensor("xb", [Q, P, QS * 4 * H], BF16, kind="ExternalInput")
    xt_d = nc.dram_tensor("xtb", [Q, P, QS * 2 * I], BF16, kind="ExternalInput")
    cst_d = nc.dram_tensor("cst", [P, CSTN], BF16, kind="ExternalInput")
    out_d = nc.dram_tensor("out", [S, OF], F32, kind="ExternalOutput")

    with tile.TileContext(nc) as tc:
        with (
            tc.tile_pool(name="consts", bufs=1) as consts,
            tc.tile_pool(name="xpool", bufs=4) as xpool,
            tc.tile_pool(name="xtpool", bufs=4) as xtpool,
            tc.tile_pool(name="work", bufs=2) as work,
            tc.tile_pool(name="ps", bufs=1, space="PSUM") as ps,
        ):
            cst = consts.tile([P, CSTN], BF16)

            def load_cst(c0, c1):
                nc.sync.dma_start(
                    out=ap(cst, [[1, c1 - c0]], off=c0),
                    in_=dram_ap(cst_d, [[CSTN, P], [1, c1 - c0]], off=c0),
                )
            w_sb = cst[:, CW:CW + 2 * OF]
            wt_sb = cst[:, CWT:CWT + 4 * H]
            id_sb = cst[:, CID:CID + P]
            md_sb = cst[:, CMD:CMD + 4 * O]
            i16_sb = cst[:, CI16:CI16 + 8]
            e8_sb = cst[:8, CE8:CE8 + P]
            c0_sb = cst[:, CC0:CC0 + O]
            c0e_sb = cst[:, CC0E:CC0E + 16]

            # PE sync anchors: PE observes foreign engine clocks via 1x1
            # transposes so walrus can elide per-instruction waits.
            anch = None  # anchors disabled; bank freed for the vo tile
            idf1 = f32view(cst[:], off=CIDF // 2)
            dirty = {}
            acol = [0]
            pending = []

            def mark(key, apv):
                dirty[key] = apv

            def pe_sync(*keys, force=False):
                # Anchors proved to over-serialize (PE waits on the globally
                # newest op of an engine, not the actual dependency); rely on
                # Tile's precise per-instruction deps instead.
                pending.clear()
                if not force:
                    return
                for k in keys:
                    if k not in dirty:
                        continue
                    d = dirty.pop(k)
                    MM_LABELS.append("anch:" + k)
                    a = nc.tensor.transpose(
                        anch[:1, acol[0]:acol[0] + 1], f32view(d),
                        idf1)
                    pending.append(a.ins)
                    acol[0] = (acol[0] + 1) % 16

            def _chain(b):
                for a in pending:
                    bass._add_dep_helper(b.ins, a, sync=False,
                                         reason="pe-anchor order")
                return b

            def T(out, in_, ident, label="", **kw):
                MM_LABELS.append(label or CUR[0] + ":T")
                return _chain(nc.tensor.transpose(out, in_, ident, **kw))

            def MM(out, lhsT, rhs, label="", **kw):
                MM_LABELS.append(label or CUR[0])
                return _chain(nc.tensor.matmul(out, lhsT, rhs, **kw))

            mark("cst", cst)
            CUR = ["init"]

            # ---- input DMAs (x natural + pre-transposed, interleaved so
            # quad q's xT lands before its first b-stage) ----
            x_sb = {}
            xt_sb = {}

            NXW = QS * 4 * H

            def load_x(q, halves=1):
                xs = xpool.tile([P, NXW], BF16, tag="x", name="xq%d" % q)
                hw_ = NXW // halves
                for h in range(halves):
                    nc.sync.dma_start(
                        out=ap(xs, [[1, hw_]], off=h * hw_),
                        in_=dram_ap(x_d, [[NXW, P], [1, hw_]],
                                    off=q * P * NXW + h * hw_),
                    )
                mark("x%d" % q, xs)
                x_sb[q] = xs

            def load_xt(q):
                xt = xtpool.tile([P, NXW], BF16, tag="xt")
                nc.sync.dma_start(
                    out=ap(xt, [[1, NXW]]),
                    in_=dram_ap(xt_d, [[NXW, P], [1, NXW]], off=q * P * NXW),
                )
                mark("xt%d" % q, xt)
                xt_sb[q] = xt

            # JIT order: small consts, x0, W, x1, xt0, x2, xt1, x3, xt2, xt3
            load_cst(CID, CSTN)
            load_x(0)
            load_cst(CW, CID)
            load_x(1)
            load_xt(0)
            load_x(2)
            load_xt(1)
            load_x(3)
            load_xt(2)
            load_xt(3)

            # PE p-state warmup: ~14 dummy matmuls on early consts while
            # the x0 DMA streams.  The PE clock ramps to full speed after
            # ~3us of continuous execution (pstate low/mid otherwise), so
            # burning the DMA-wait keeps the real A-stage at full clock.
            ps_w = ps.tile([P, H], F32, tag="y", name="ps_warm")
            for _wi in range(16):
                MM(ps_w[:, :H], id_sb,
                   ap(cst, [[1, H]], off=CID), label="warm")

            cT = {q: None for q in range(Q)}
            fin = {}

            def mk_chunks(q):
                """Per-quad list of emission chunks; wavefront-interleaved
                across quads so PE always has another quad's work during
                cross-engine chain latencies."""
                st = {}
                chunks = []
                for t in range(NIT):
                    chunks.extend(stage_fns(q, t, st))
                return chunks

            def stage_fns(q, t, st):
                last = (t == NIT - 1)

                def A():
                    CUR[0] = "A.q%d.t%d" % (q, t)
                    pe_sync("cst", "x%d" % q, "dve")
                    ps_y = ps.tile([P, H], F32, tag="y")
                    if t == 0:
                        # uniform c: y0 is o-independent -> [4, 256] rows,
                        # one 16-MM accumulation group via c0e selectors
                        for s in range(QS):
                            for ic in range(4):
                                MM(
                                    ps_y[0:4, :],
                                    ap(c0e_sb, [[1, 4]], off=4 * s),
                                    ap(x_sb[q], [[1, H]],
                                       off=s * 4 * H + ic * H),
                                    start=(s == 0 and ic == 0),
                                    stop=(s == 3 and ic == 3),
                                )
                        ysb = work.tile([4, H], BF16, tag="ysb0", bufs=4)
                        nc.scalar.activation(ysb[:], ps_y[0:4, :], AF.Copy)
                    else:
                        for s in range(QS):
                            for ic in range(4):
                                MM(
                                    ps_y[32 * s:32 * s + 32, :],
                                    ap(cT[q], [[1, O]], off=ic * P + s * O),
                                    ap(x_sb[q], [[1, H]],
                                       off=s * 4 * H + ic * H),
                                    start=(ic == 0),
                                    stop=(ic == 3),
                                    tile_position=(0, 32 * s),
                                )
                        ysb = work.tile([P, H], BF16, tag="ysb", bufs=4)
                        for hh in range(2):
                            nc.scalar.activation(
                                ysb[:, hh * P:(hh + 1) * P],
                                ps_y[:, hh * P:(hh + 1) * P], AF.Copy)
                    mark("act", ysb)
                    st["ysb"] = ysb

                def Bc():
                    CUR[0] = "Bc.q%d.t%d" % (q, t)
                    pe_sync("act")
                    ps_yt = ps.tile([P, 4 * P], BF16, tag="ytb")
                    if t == 0:
                        for hc in range(2):
                            T(ps_yt[:, hc * 4:(hc + 1) * 4],
                              st["ysb"][:4, hc * P:(hc + 1) * P],
                              id_sb[:4, :4])
                        ytsb = work.tile([P, 8], BF16, tag="ytsb0", bufs=4)
                        nc.scalar.activation(ytsb[:], ps_yt[:, :8], AF.Copy)
                    else:
                        for hc in range(2):
                            T(ps_yt[:, hc * P:(hc + 1) * P],
                              st["ysb"][:, hc * P:(hc + 1) * P], id_sb)
                        ytsb = work.tile([P, 2 * P], BF16, tag="ytsb", bufs=4)
                        nc.vector.tensor_copy(ytsb[:], ps_yt[:, :2 * P])
                    mark("act", ytsb)
                    st["ytsb"] = ytsb

                def C():
                    CUR[0] = "C.q%d.t%d" % (q, t)
                    pe_sync("dve")
                    if t == 0:
                        ps_vf = ps.tile([P, 16], F32, tag="vf",
                                        name="ps_vf0")
                        for g in range(4):
                            for hc in range(2):
                                MM(
                                    ps_vf[:, g * 4:(g + 1) * 4],
                                    ap(w_sb, [[1, P]], off=hc * OF + g * P),
                                    ap(st["ytsb"], [[1, 4]], off=hc * 4),
                                    start=(hc == 0),
                                    stop=(hc == 1),
                                )
                        vr = work.tile([P, 16], F32, tag="vr", bufs=4)
                        nc.vector.tensor_copy(vr[:], ps_vf[:])
                    else:
                        ps_vf = ps.tile([P, 4 * P], F32, tag="vf")
                        for g in range(4):
                            for hc in range(2):
                                MM(
                                    ps_vf[:, g * P:(g + 1) * P],
                                    ap(w_sb, [[1, P]], off=hc * OF + g * P),
                                    ap(st["ytsb"], [[1, P]], off=hc * P),
                                    start=(hc == 0),
                                    stop=(hc == 1),
                                )
                        msk = work.tile([P, 4 * P], BF16, tag="msk")
                        nc.vector.tensor_mul(
                            ap(msk, [[P, 4], [O, QS], [1, O]]),
                            ap(ps_vf, [[P, 4], [O, QS], [1, O]]),
                            ap(md_sb, [[O, 4], [0, QS], [1, O]]),
                        )
                        st["msk"] = msk
                        vr = work.tile([P, 16], F32, tag="vr", bufs=4)
                        nc.vector.reduce_sum(
                            out=vr[:],
                            in_=ap(msk, [[O, 16], [1, O]]),
                            axis=AX.X,
                        )
                    vrsq = work.tile([P, 16], BF16, tag="vrsq", bufs=4)
                    nc.vector.tensor_mul(vrsq[:], vr[:], vr[:])
                    mark("dve", vrsq)
                    st["vr"] = vr
                    st["vrsq"] = vrsq

                def D():
                    CUR[0] = "D.q%d.t%d" % (q, t)
                    pe_sync("dve")
                    mf = ps.tile([P, 32], F32, tag="mf")
                    if t > 0:
                        MM(mf[:8, :16], i16_sb,
                           ap(st["msk"], [[1, 16]]), label="ping")
                    MM(mf[:8, :16], i16_sb, st["vrsq"][:])
                    lnm = work.tile([8, 16], F32, tag="lnm", bufs=4)
                    nc.scalar.activation(lnm[:], mf[:8, :16], AF.Ln)
                    s0 = work.tile([8, 16], F32, tag="s0", bufs=4)
                    nc.scalar.activation(s0[:], lnm[:], AF.Exp, scale=0.5)
                    st["s0"] = s0
                    mark("act", s0)
                    onep = work.tile([8, 16], F32, tag="onep", bufs=4)
                    nc.vector.tensor_scalar_add(onep[:], mf[:8, :16], 1.0)
                    rp = work.tile([8, 16], F32, tag="rp", bufs=4)
                    nc.vector.reciprocal(rp[:], onep[:])
                    facb = work.tile([8, 16], BF16, tag="facb", bufs=4)
                    nc.vector.tensor_mul(facb[:], s0[:], rp[:])
                    mark("dve", facb)
                    st["mf"] = mf
                    st["facb"] = facb

                def E():
                    CUR[0] = "E.q%d.t%d" % (q, t)
                    mf = st["mf"]
                    pe_sync("dve", "act")
                    MM(mf[0:1, 16:32],
                       nc.const_aps.tensor(1.0, (8, 1)),
                       st["s0"][:8, :], label="pingE")
                    MM(mf[:, 16:32], e8_sb, st["facb"][:8, :])
                    if not last:
                        vsq = work.tile([P, 16], BF16, tag="vsq", bufs=4)
                        nc.vector.tensor_mul(vsq[:], st["vr"][:],
                                             mf[:, 16:32])
                        mark("dve", vsq)
                        vp = work.tile([P, 4 * P], BF16, tag="vp", bufs=4)
                        nc.vector.tensor_mul(
                            ap(vp, [[P, 4], [O, QS], [1, O]]),
                            ap(vsq, [[4, 4], [1, QS], [0, O]]),
                            ap(md_sb, [[O, 4], [0, QS], [1, O]]),
                        )
                        mark("pool", vp)
                        st["vp"] = vp
                    else:
                        # all quads write one [128, (q, s, g)] tile; single
                        # transpose + copy + cast-DMA at the last quad
                        if "vsq_all" not in fin:
                            fin["vsq_all"] = work.tile([P, 64], BF16,
                                                       tag="vsq_all", bufs=1,
                                                       name="vsq_all")
                        nc.vector.tensor_mul(
                            ap(fin["vsq_all"], [[1, 4], [4, 4]], off=16 * q),
                            ap(st["vr"], [[4, 4], [1, 4]]),
                            ap(mf, [[4, 4], [1, 4]], off=16),
                        )
                        fin["done"] = fin.get("done", 0) + 1
                        if fin["done"] == 2:
                            # first output half transposed early, hidden
                            # under the remaining quads' t2 compute
                            fin["ps_vo"] = ps.tile([P, P], BF16, tag="y",
                                                   name="ps_vo")
                            T(fin["ps_vo"][0:32, :P],
                              fin["vsq_all"][:, 0:32], id_sb, label="OUT:T1")
                        if fin["done"] == Q:
                            ps_vo = fin["ps_vo"]
                            T(ps_vo[32:64, :P], fin["vsq_all"][:, 32:64],
                              id_sb, label="OUT:T2")
                            # f32 cast in the DVE copy so the output DMA can
                            # use the sync engine's HW DGE (no gpsimd SW-DGE
                            # cast overhead on the tail)
                            vo = work.tile([64, P], F32, tag="vosb")
                            nc.vector.tensor_copy(vo[:], ps_vo[:64, :P])
                            nc.sync.dma_start(
                                out=dram_ap(out_d, [[P, 64], [1, P]]),
                                in_=vo[:],
                            )

                def Fc():
                    CUR[0] = "Fc.q%d.t%d" % (q, t)
                    pe_sync("dve")
                    ps_zt = ps.tile([P, 2 * P], F32, tag="zt")
                    for hc in range(2):
                        for m in range(4):
                            MM(
                                ps_zt[:, hc * P:(hc + 1) * P],
                                ap(wt_sb, [[1, P]], off=m * H + hc * P),
                                ap(st["vp"], [[1, P]], off=m * P),
                                start=(m == 0),
                                stop=(m == 3),
                            )
                    ztsb = work.tile([P, 2 * P], BF16, tag="ztsb", bufs=4)
                    for hh in range(2):
                        nc.scalar.activation(
                            ztsb[:, hh * P:(hh + 1) * P],
                            ps_zt[:, hh * P:(hh + 1) * P], AF.Copy)
                    mark("act", ztsb)
                    st["ztsb"] = ztsb

                def G():
                    CUR[0] = "G.q%d.t%d" % (q, t)
                    pe_sync("act", "xt%d" % q)
                    ps_b = ps.tile([P, I], F32, tag="b", bufs=2)
                    for s in range(QS):
                        for hc in range(2):
                            MM(
                                ps_b[32 * s:32 * s + 32, :],
                                ap(st["ztsb"], [[1, O]], off=hc * P + s * O),
                                ap(xt_sb[q], [[1, I]], off=s * 2 * I + hc * I),
                                start=(hc == 0),
                                stop=(hc == 1),
                                tile_position=(0, 32 * s),
                            )
                    eb = work.tile([P, I], BF16, tag="eb", bufs=4)
                    for hh in range(2):
                        nc.scalar.activation(
                            eb[:, hh * 2 * P:(hh + 1) * 2 * P],
                            ps_b[:, hh * 2 * P:(hh + 1) * 2 * P], AF.Exp)
                    mark("act", eb)
                    st["eb"] = eb

                def Hc():
                    CUR[0] = "Hc.q%d.t%d" % (q, t)
                    pe_sync("act")
                    ps_ebt = ps.tile([P, 4 * P], BF16, tag="ebt")
                    for ic in range(4):
                        T(ps_ebt[:, ic * P:(ic + 1) * P],
                          st["eb"][:, ic * P:(ic + 1) * P], id_sb)
                    ssum = work.tile([P, 16], F32, tag="ssum")
                    nc.vector.reduce_sum(
                        out=ssum[:],
                        in_=ap(ps_ebt, [[O, 16], [1, O]]),
                        axis=AX.X,
                    )
                    rs = work.tile([P, 16], F32, tag="rs")
                    nc.vector.reciprocal(rs[:], ssum[:])
                    ct = work.tile([P, 4 * P], BF16, tag="ct%d" % q, bufs=1)
                    nc.vector.tensor_mul(
                        ap(ct, [[O, 16], [1, O]]),
                        ap(ps_ebt, [[O, 16], [1, O]]),
                        ap(rs, [[1, 16], [0, O]]),
                    )
                    mark("dve", ct)
                    cT[q] = ct

                if last:
                    return [A, Bc, C, D, E]
                return [A, Bc, C, D, E, Fc, G, Hc]

            all_chunks = {q: mk_chunks(q) for q in range(Q)}
            L = len(all_chunks[0])
            SKEW = 2
            for k in range(L + SKEW * (Q - 1)):
                for q in reversed(range(Q)):
                    c = k - SKEW * q
                    if 0 <= c < L:
                        all_chunks[q][c]()

    if split_waits:
        _split_fat_waits(nc)
    return nc


def _split_fat_waits(nc, maxw=1):
    """Walrus caps sync waits per instruction; split overflow onto extra
    same-engine Drain instructions inserted just before the offender."""
    nsplit = 0
    for blk in nc.m.functions[0].blocks:
        new_insts = []
        for inst in blk.instructions:
            si = getattr(inst, "sync_info", None)
            w = list(si.on_wait) if si is not None and si.on_wait else []
            if len(w) > maxw:
                for k in range(0, len(w) - maxw, maxw):
                    d = mybir.InstDrain(name="I-waitsplit-%d" % nsplit,
                                        ins=[], outs=[])
                    nsplit += 1
                    d.engine = inst.engine
                    d.sync_info = mybir.SyncInfo(on_wait=w[k:k + maxw],
                                                 on_update=[])
                    new_insts.append(d)
                si.on_wait = w[len(w) - maxw:]
            new_insts.append(inst)
        blk.instructions[:] = new_insts
    return nc


_NC_CACHE = None


def make_cst(Wn):
    """bf16 constant blob [128, CSTN] matching the device-side layout."""
    cst = np.zeros((P, CSTN), np.float32)
    cst[:, CW:CW + 2 * OF] = (
        Wn.reshape(2, P, OF).transpose(1, 0, 2).reshape(P, 2 * OF))
    cst[:, CWT:CWT + 4 * H] = (
        Wn.T.reshape(4, P, H).transpose(1, 0, 2).reshape(P, 4 * H))
    cst[:, CID:CID + P] = np.eye(P, dtype=np.float32)
    for p in range(P):
        for g in range(4):
            cst[p, CMD + g * O + g * 8 + p // 16] = 1.0
    cst[np.arange(P), CI16 + np.arange(P) // 16] = 1.0
    for j in range(8):
        cst[j, CE8 + 16 * j:CE8 + 16 * (j + 1)] = 1.0
    cst[:, CC0:CC0 + O] = 1.0 / O
    for s in range(4):
        cst[:, CC0E + 4 * s + s] = 1.0 / O
    out = cst.astype(ml_dtypes.bfloat16)
    # bf16 pair (0.0, 1.0) little-endian == f32 1.0 when viewed 4-byte
    out[:, CIDF] = ml_dtypes.bfloat16(0.0)
    out[:, CIDF + 1] = ml_dtypes.bfloat16(1.0)
    return out


def make_in_maps(x, W):
    x = np.asarray(x, dtype=np.float32)
    Wn = np.asarray(W, dtype=np.float32).reshape(H, OF)
    cst = make_cst(Wn)
    xq = x.astype(ml_dtypes.bfloat16).reshape(NCORES, Q, QS, 4, P, H)
    # xb[c, q, p, (s, ic, h)] = x[c, 4q+s, 128ic+p, h]
    xb = np.ascontiguousarray(xq.transpose(0, 1, 4, 2, 3, 5)).reshape(
        NCORES, Q, P, QS * 4 * H)
    # xtb[c, q, p, (s, hc, i)] = x[c, 4q+s, i, 128hc+p]
    xth = x.astype(ml_dtypes.bfloat16).reshape(NCORES, Q, QS, I, 2, P)
    xtb = np.ascontiguousarray(xth.transpose(0, 1, 5, 2, 4, 3)).reshape(
        NCORES, Q, P, QS * 2 * I)
    return [
        {"xb": xb[c], "xtb": xtb[c], "cst": cst}
        for c in range(NCORES)
    ]


def kernel(x: np.ndarray, W: np.ndarray) -> np.ndarray:
    global _NC_CACHE
    if _NC_CACHE is None:
        _NC_CACHE = build_program()
    in_maps = make_in_maps(x, W)
    res = run_bass_kernel_spmd(_NC_CACHE, in_maps, core_ids=list(range(NCORES)))
    out = np.stack([res.results[c]["out"] for c in range(NCORES)])
    return out.reshape(B, O, F)

